# revision 1
# baseline (speedup 1.0000x reference)
"""Trainium2 Bass kernel for nn_CrossAttention (dense_transformer).

Reference computation (per batch b, per stream s in {1,2}):
    q_s   = heads(x_s)                      # [H, N, D] slices of x_s
    kv_s  = x_s @ Wkv_s -> k_s, v_s         # [N, C] each
    gate_s= sigmoid(relu(x_s @ w1 + b1) @ w2 + b2)
    ctx_s = softmax_d( scale * k_s^T @ (v_s * gate_s) )   # [H, D, D], softmax over d
    o_1   = q_1 @ ctx_2 ; o_2 = q_2 @ ctx_1  (cross)

Sharding: 8 cores = (stream s, batch b) pairs.  Core (s, b) projects
x_s[b] (kv + gate + ctx_s[b]) and then computes the OTHER stream's
output o_{1-s}[b] = q_{1-s}[b] @ softmax(ctx_s[b]).  No cross-core
communication; host concatenates outputs.
"""

import numpy as np
from contextlib import ExitStack

N = 4096
C = 1024
H = 16
D = 64
SCALE = D ** (-0.5)
NCH = N // 128       # 32 n-chunks of 128 rows
KCH = C // 128       # 8 contraction chunks
F32 = None           # set lazily (mybir import)

_CACHE = {}


def _build_program(with_bias):
    """Build the SPMD Bass program (same for all 8 cores)."""
    import concourse.bass as bass
    import concourse.bacc as bacc
    import concourse.tile as tile
    import concourse.mybir as mybir

    F32 = mybir.dt.float32
    F32R = mybir.dt.float32r
    BF16 = mybir.dt.bfloat16
    AF = mybir.ActivationFunctionType

    nc = bacc.Bacc("TRN2", target_bir_lowering=False, debug=False, num_devices=8)

    xp = nc.dram_tensor("xp", [N, C], F32R, kind="ExternalInput").ap()
    xq = nc.dram_tensor("xq", [N, C], F32R, kind="ExternalInput").ap()
    wkv = nc.dram_tensor("wkv", [C, 2 * C], F32R, kind="ExternalInput").ap()
    w1 = nc.dram_tensor("w1", [C, C], F32R, kind="ExternalInput").ap()
    b1 = nc.dram_tensor("b1", [C], F32, kind="ExternalInput").ap()
    w2 = nc.dram_tensor("w2", [C, C], F32R, kind="ExternalInput").ap()
    b2 = nc.dram_tensor("b2", [C], F32R, kind="ExternalInput").ap()
    ident = nc.dram_tensor("ident", [128, 128], F32R, kind="ExternalInput").ap()
    identb = nc.dram_tensor("identb", [128, 128], BF16, kind="ExternalInput").ap()
    o = nc.dram_tensor("o", [N, C], F32R, kind="ExternalOutput").ap()


    with tile.TileContext(nc) as tc, ExitStack() as ctx:
        # ---------- persistent pools ----------
        cpool = ctx.enter_context(tc.tile_pool(name="consts", bufs=1))
        ident_sb = cpool.tile([128, 128], F32R, name="ident_sb")
        nc.sync.dma_start(ident_sb, ident)
        identf = cpool.tile([128, 128], F32, name="identf")
        nc.vector.tensor_copy(identf, ident_sb)
        identb_sb = cpool.tile([128, 128], BF16, name="identb_sb")
        nc.sync.dma_start(identb_sb, identb)
        b1_sb = cpool.tile([128, 8], F32, name="b1_sb")  # b1_sb[p, m] = b1[m*128+p]
        nc.sync.dma_start(b1_sb, b1.rearrange("(m p) -> p m", p=128))
        if with_bias:
            ones_sb = cpool.tile([1, 128], F32, name="ones_sb")
            nc.vector.memset(ones_sb, 1.0)
            ones_r = cpool.tile([1, 128], F32R, name="ones_r")
            nc.vector.tensor_copy(ones_r, ones_sb)
            b2_r = cpool.tile([1, C], F32R, name="b2_r")
            nc.sync.dma_start(b2_r, b2.rearrange("(one f) -> one f", one=1))

        acc_pool = ctx.enter_context(tc.tile_pool(name="ctxacc", bufs=1))
        # ctxT accumulator on partitions 0-63: head h -> cols [h*64, h*64+64), layout [e, d]
        ctx_acc = acc_pool.tile([64, 1024], F32, name="ctx_acc")
        nc.vector.memset(ctx_acc, 0.0)

        spool = ctx.enter_context(tc.tile_pool(name="spairs", bufs=1))
        spairs = [spool.tile([128, 128], BF16, name=f"spair{j}") for j in range(8)]

        dpool = ctx.enter_context(tc.tile_pool(name="scratch", bufs=1, space="DRAM"))
        g_dram = dpool.tile([N, C], F32, name="g_dram")
        xpT_dram = dpool.tile([C, N], F32R, name="xpT_dram")

        # =========================================================
        # Phase A1: gate MLP for all n; also builds/spills xp^T.
        #   gate1 transposed-out: hT[m-tile, n] = (xp @ w1).T  (w1 stationary)
        #   gate2 normal-out:     g[n, :] = sigmoid(h @ w2 + b2)  (hT stationary)
        # =========================================================
        with ExitStack() as a1:
            wpool = a1.enter_context(tc.tile_pool(name="a1w", bufs=1))
            w1_sb = wpool.tile([128, 8, C], F32R, name="w1_sb")  # [p, k, col]
            nc.sync.dma_start(w1_sb, w1.rearrange("(k p) m -> p k m", p=128))
            w2_sb = wpool.tile([128, 8, C], F32R, name="w2_sb")
            nc.sync.dma_start(w2_sb, w2.rearrange("(k p) m -> p k m", p=128))

            ht_pool = a1.enter_context(tc.tile_pool(name="a1ht", bufs=1))
            gout_pool = a1.enter_context(tc.tile_pool(name="a1g", bufs=1))
            g1ps_pool = a1.enter_context(
                tc.tile_pool(name="a1g1ps", bufs=4, space="PSUM")
            )
            g2ps_pool = a1.enter_context(
                tc.tile_pool(name="a1g2ps", bufs=2, space="PSUM")
            )

            def emit_transposes_g1(sb, xpt_pool, xin_pool, trps_pool):
                xpt = [
                    xpt_pool.tile([128, 1024], F32R, name=f"xpt{j}", tag=f"xpt{j}", bufs=1)
                    for j in range(8)
                ]
                for grp in range(2):  # 512-row halves
                    xins = []
                    for c4 in range(4):
                        xin = xin_pool.tile([128, C], F32R, name="xin", tag="xin")
                        nch = sb * 8 + grp * 4 + c4
                        nc.sync.dma_start(xin, xp[nch * 128:(nch + 1) * 128, :])
                        xins.append(xin)
                    for j in range(8):
                        tps = trps_pool.tile([128, 512], F32R, name="tps", tag="tps")
                        for c4 in range(4):
                            nc.tensor.transpose(
                                tps[:, c4 * 128:(c4 + 1) * 128],
                                xins[c4][:, j * 128:(j + 1) * 128],
                                ident_sb,
                            )
                        if j % 2 == 0:
                            nc.vector.tensor_copy(
                                xpt[j][:, grp * 512:(grp + 1) * 512], tps
                            )
                        else:
                            nc.scalar.copy(
                                xpt[j][:, grp * 512:(grp + 1) * 512], tps
                            )
                # spill xp^T
                for j in range(8):
                    nc.sync.dma_start(
                        xpT_dram[j * 128:(j + 1) * 128, sb * 1024:(sb + 1) * 1024],
                        xpt[j],
                    )
                # gate1 transposed: hT[m] = sum_k w1[k,m].T @ xpT[k]
                hts = [
                    ht_pool.tile([128, 1024], F32R, name=f"ht{m}", tag=f"ht{m}", bufs=2)
                    for m in range(8)
                ]
                for m in range(8):
                    pss = [
                        g1ps_pool.tile([128, 512], F32, name="g1ps", tag="g1ps")
                        for _ in range(2)
                    ]
                    for k in range(8):
                        lhs = w1_sb[:, k, m * 128:(m + 1) * 128]
                        for half in range(2):
                            nc.tensor.matmul(
                                pss[half],
                                lhs,
                                xpt[k][:, half * 512:(half + 1) * 512],
                                start=(k == 0),
                                stop=(k == 7),
                            )
                    for half in range(2):
                        nc.scalar.activation(
                            hts[m][:, half * 512:(half + 1) * 512],
                            pss[half],
                            AF.Relu,
                            bias=b1_sb[:, m:m + 1],
                        )
                return hts

            def emit_g2(sb, hts):
                for c in range(8):
                    nch = sb * 8 + c
                    gt = gout_pool.tile([128, C], F32, name="gt", tag="gt")
                    for t in range(2):
                        ps2 = g2ps_pool.tile([128, 512], F32, name="g2ps", tag="g2ps")
                        for k in range(8):
                            nc.tensor.matmul(
                                ps2,
                                hts[k][:, c * 128:(c + 1) * 128],
                                w2_sb[:, k, t * 512:(t + 1) * 512],
                                start=(k == 0),
                                stop=(k == 7 and not with_bias),
                            )
                        if with_bias:
                            nc.tensor.matmul(
                                ps2,
                                ones_r,
                                b2_r[:, t * 512:(t + 1) * 512],
                                start=False,
                                stop=True,
                            )
                        nc.scalar.activation(
                            gt[:, t * 512:(t + 1) * 512], ps2, AF.Sigmoid
                        )
                    nc.sync.dma_start(g_dram[nch * 128:(nch + 1) * 128, :], gt)

            with ExitStack() as a1inner:
                xpt_pool_i = a1inner.enter_context(tc.tile_pool(name="a1xpt", bufs=1))
                xin_pool_i = a1inner.enter_context(tc.tile_pool(name="a1xin", bufs=6))
                trps_pool_i = a1inner.enter_context(
                    tc.tile_pool(name="a1trps", bufs=2, space="PSUM")
                )
                for sb in range(3):
                    hts = emit_transposes_g1(sb, xpt_pool_i, xin_pool_i, trps_pool_i)
                    emit_g2(sb, hts)
                hts3 = emit_transposes_g1(3, xpt_pool_i, xin_pool_i, trps_pool_i)
            # xpt/xin/trps pools are now closed: A2's wkv tile will alias their
            # space, so its DMA can start while gate2(sb3) still runs on PE.
            emit_g2(3, hts3)

        # early phase-B pools: transposing xq is independent of A2/SM, so give
        # it non-aliased space and let the scheduler overlap it with A2/SM.
        bxin_pool = ctx.enter_context(tc.tile_pool(name="bxin", bufs=5))
        bxqt_pool = ctx.enter_context(tc.tile_pool(name="bxqt", bufs=1))
        btrps_early_pool = ctx.enter_context(
            tc.tile_pool(name="btrpse", bufs=2, space="PSUM")
        )
        bxqt_tiles = {}

        def emit_xq_transposes(blk):
            xins = []
            for c4 in range(4):
                xin = bxin_pool.tile([128, C], F32R, name="bxin", tag="bxin")
                nch = blk * 4 + c4
                nc.sync.dma_start(xin, xq[nch * 128:(nch + 1) * 128, :])
                xinb = bxin_pool.tile([128, C], BF16, name="bxinb", tag="bxinb")
                if c4 % 2 == 0:
                    nc.vector.tensor_copy(xinb, xin)
                else:
                    nc.scalar.copy(xinb, xin)
                xins.append(xinb)
            xqts = [
                bxqt_pool.tile(
                    [128, 512], BF16, name=f"xqt{j}", tag=f"xqt{j}", bufs=3
                )
                for j in range(8)
            ]
            for j in range(8):
                tps = btrps_early_pool.tile(
                    [128, 512], BF16, name="btps", tag="btps"
                )
                for c4 in range(4):
                    nc.tensor.transpose(
                        tps[:, c4 * 128:(c4 + 1) * 128],
                        xins[c4][:, j * 128:(j + 1) * 128],
                        identb_sb,
                    )
                if j % 2 == 0:
                    nc.vector.tensor_copy(xqts[j], tps)
                else:
                    nc.scalar.copy(xqts[j], tps)
            bxqt_tiles[blk] = xqts

        emit_xq_transposes(0)
        emit_xq_transposes(1)
        emit_xq_transposes(2)

        # =========================================================
        # Phase A2: kv projection + ctx accumulation.
        #   kv normal-out (xpT stationary); ctxT_h += vg_h.T @ k_h
        # =========================================================
        with ExitStack() as a2:
            wkv_pool = a2.enter_context(tc.tile_pool(name="a2w", bufs=1))
            wkv_sb = wkv_pool.tile([128, 8, 2 * C], F32R, name="wkv_sb")
            nc.sync.dma_start(wkv_sb, wkv.rearrange("(k p) m -> p k m", p=128))

            xpt_in_pool = a2.enter_context(tc.tile_pool(name="a2xpt", bufs=3))
            gin_pool = a2.enter_context(tc.tile_pool(name="a2gin", bufs=3))
            k_pool = a2.enter_context(tc.tile_pool(name="a2k", bufs=2))
            v_pool = a2.enter_context(tc.tile_pool(name="a2v", bufs=2))
            vg_pool = a2.enter_context(tc.tile_pool(name="a2vg", bufs=2))
            kvps_pool = a2.enter_context(
                tc.tile_pool(name="a2kvps", bufs=4, space="PSUM")
            )
            ctps_pool = a2.enter_context(
                tc.tile_pool(name="a2ctps", bufs=1, space="PSUM")
            )

            for nch in range(NCH):
                xpt_in = xpt_in_pool.tile([128, C], F32R, name="xpt_in", tag="xpt_in")
                nc.sync.dma_start(
                    xpt_in,
                    xpT_dram.rearrange("(k p) n -> p k n", p=128)[
                        :, :, nch * 128:(nch + 1) * 128
                    ],
                )
                gin = gin_pool.tile([128, C], F32, name="gin", tag="gin")
                nc.sync.dma_start(gin, g_dram[nch * 128:(nch + 1) * 128, :])

                kvps = [
                    kvps_pool.tile([128, 512], F32, name="kvps", tag="kvps")
                    for _ in range(4)
                ]
                for k in range(8):
                    lhs = xpt_in[:, k * 128:(k + 1) * 128]
                    for t in range(4):
                        nc.tensor.matmul(
                            kvps[t],
                            lhs,
                            wkv_sb[:, k, t * 512:(t + 1) * 512],
                            start=(k == 0),
                            stop=(k == 7),
                        )
                k_sb = k_pool.tile([128, C], F32R, name="k_sb", tag="k_sb")
                v_sb = v_pool.tile([128, C], F32, name="v_sb", tag="v_sb")
                nc.scalar.copy(k_sb[:, 0:512], kvps[0])
                nc.scalar.copy(k_sb[:, 512:1024], kvps[1])
                nc.vector.tensor_copy(v_sb[:, 0:512], kvps[2])
                nc.vector.tensor_copy(v_sb[:, 512:1024], kvps[3])
                vg = vg_pool.tile([128, C], F32R, name="vg", tag="vg")
                nc.vector.tensor_mul(vg, v_sb, gin)

                ctp = ctps_pool.tile([64, 1024], F32, name="ctp", tag="ctp")
                for h in range(H):
                    nc.tensor.matmul(
                        ctp[:, h * D:(h + 1) * D],
                        vg[:, h * D:(h + 1) * D],
                        k_sb[:, h * D:(h + 1) * D],
                        start=True,
                        stop=True,
                        skip_group_check=True,
                    )
                nc.vector.tensor_add(ctx_acc, ctx_acc, ctp)

        # =========================================================
        # Softmax over d (free dim of ctxT) + build block-diag S pairs
        # =========================================================
        with ExitStack() as sm:
            smp = sm.enter_context(tc.tile_pool(name="smpool", bufs=1))
            smps = sm.enter_context(tc.tile_pool(name="smps", bufs=2, space="PSUM"))
            maxs = smp.tile([64, 16], F32, name="maxs")
            nc.vector.tensor_reduce(
                maxs,
                ctx_acc.rearrange("p (b d) -> p b d", b=16),
                axis=mybir.AxisListType.X,
                op=mybir.AluOpType.max,
            )
            cmx = smp.tile([64, 1024], F32, name="cmx")
            nc.vector.tensor_sub(
                cmx.rearrange("p (h d) -> p h d", h=16),
                ctx_acc.rearrange("p (h d) -> p h d", h=16),
                maxs.unsqueeze(-1).broadcast_to([64, 16, 64]),
            )
            et = smp.tile([64, 1024], F32, name="et")
            nc.scalar.activation(et, cmx, AF.Exp, scale=float(SCALE))
            sums = smp.tile([64, 16], F32, name="sums")
            nc.vector.tensor_reduce(
                sums,
                et.rearrange("p (b d) -> p b d", b=16),
                axis=mybir.AxisListType.X,
                op=mybir.AluOpType.add,
            )
            recs = smp.tile([64, 16], F32, name="recs")
            nc.vector.reciprocal(recs, sums)
            st = smp.tile([64, 1024], F32, name="st")
            nc.vector.tensor_mul(
                st.rearrange("p (h d) -> p h d", h=16),
                et.rearrange("p (h d) -> p h d", h=16),
                recs.unsqueeze(-1).broadcast_to([64, 16, 64]),
            )
            # st: softmaxed ctxT [e, d] per head at cols h*64.  Transposing the
            # side-by-side pair [ctxT_2j | ctxT_2j+1] ([64, 128]) gives
            # [S_2j stacked above S_2j+1] ([128, 64]); scatter to block-diag.
            zero_sb = smp.tile([128, 128], BF16, name="zero_sb")
            nc.vector.memset(zero_sb, 0.0)
            for j in range(8):
                tp = smps.tile([128, 64], F32, name="smtp", tag="smtp")
                nc.tensor.transpose(
                    tp, st[:, (2 * j) * 64:(2 * j + 2) * 64], identf[0:64, 0:64]
                )
                nc.vector.tensor_copy(spairs[j], zero_sb)
                nc.vector.tensor_copy(spairs[j][0:64, 0:64], tp[0:64, :])
                nc.vector.tensor_copy(spairs[j][64:128, 64:128], tp[64:128, :])

        # =========================================================
        # Phase B: o[nchunk, j*128:(j+1)*128] = (xqT_j_chunk).T @ spair_j
        # (normal orientation directly; no back-transposes)
        # =========================================================
        with ExitStack() as pb:
            oout_pool = pb.enter_context(tc.tile_pool(name="bo", bufs=6))
            bops_pool = pb.enter_context(tc.tile_pool(name="bops", bufs=4, space="PSUM"))

            for blk in range(8):
                if blk + 3 < 8:
                    emit_xq_transposes(blk + 3)
                xqts = bxqt_tiles.pop(blk)
                oouts = [
                    oout_pool.tile([128, C], F32R, name="oo", tag="oo")
                    for _ in range(4)
                ]
                for c4 in range(4):
                    for half in range(2):
                        ops = bops_pool.tile([128, 512], F32, name="ops", tag="ops")
                        for jj in range(4):
                            j = half * 4 + jj
                            nc.tensor.matmul(
                                ops[:, jj * 128:(jj + 1) * 128],
                                xqts[j][:, c4 * 128:(c4 + 1) * 128],
                                spairs[j],
                                start=True,
                                stop=True,
                                skip_group_check=True,
                            )
                        if half == 0:
                            nc.vector.tensor_copy(
                                oouts[c4][:, half * 512:(half + 1) * 512], ops
                            )
                        else:
                            nc.scalar.copy(
                                oouts[c4][:, half * 512:(half + 1) * 512], ops
                            )
                for c4 in range(4):
                    nch = blk * 4 + c4
                    nc.sync.dma_start(o[nch * 128:(nch + 1) * 128, :], oouts[c4])

    nc.compile()
    return nc


def _get_program(with_bias=False):
    key = ("nc", bool(with_bias))
    if key not in _CACHE:
        _CACHE[key] = _build_program(with_bias)
    return _CACHE[key]


def make_in_maps(x1, x2, Wkv1, Wkv2, g1_w1, g1_b1, g1_w2, g1_b2,
                 g2_w1, g2_b1, g2_w2, g2_b2):
    """Core (s, b): cores 0-3 = (s=0, b), cores 4-7 = (s=1, b)."""
    import ml_dtypes
    ident = np.eye(128, dtype=np.float32)
    identb = np.eye(128, dtype=ml_dtypes.bfloat16)
    asf = np.ascontiguousarray
    in_maps = []
    for core in range(8):
        s, b = core // 4, core % 4
        if s == 0:
            m = dict(xp=asf(x1[b]), xq=asf(x2[b]), wkv=asf(Wkv1),
                     w1=asf(g1_w1), b1=asf(g1_b1), w2=asf(g1_w2), b2=asf(g1_b2))
        else:
            m = dict(xp=asf(x2[b]), xq=asf(x1[b]), wkv=asf(Wkv2),
                     w1=asf(g2_w1), b1=asf(g2_b1), w2=asf(g2_w2), b2=asf(g2_b2))
        m["ident"] = ident
        m["identb"] = identb
        in_maps.append(m)
    return in_maps


def kernel(x1, x2, Wkv1, Wkv2, g1_w1, g1_b1, g1_w2, g1_b2,
           g2_w1, g2_b1, g2_w2, g2_b2, _runner=None):
    """Full-input entry point.  Returns (o1, o2), each [4, 4096, 1024] f32."""
    from concourse.bass_utils import run_bass_kernel_spmd

    args = [np.asarray(a, dtype=np.float32) for a in
            (x1, x2, Wkv1, Wkv2, g1_w1, g1_b1, g1_w2, g1_b2,
             g2_w1, g2_b1, g2_w2, g2_b2)]
    with_bias = bool(np.any(args[7]) or np.any(args[11]))  # g1_b2, g2_b2
    nc = _get_program(with_bias)
    in_maps = make_in_maps(*args)
    if _runner is None:
        res = run_bass_kernel_spmd(nc, in_maps, core_ids=list(range(8)))
        results = res.results
    else:
        results = _runner(nc, in_maps)

    B = x1.shape[0]
    o1 = np.empty((B, N, C), dtype=np.float32)
    o2 = np.empty((B, N, C), dtype=np.float32)
    for core in range(8):
        s, b = core // 4, core % 4
        out = results[core]["o"]
        if s == 0:
            o2[b] = out   # core projected x1 -> ctx1 -> o2 = q2 @ ctx1
        else:
            o1[b] = out
    return (o1, o2)



# revision 7
# speedup vs baseline: 1.2679x; 1.2679x over previous
"""Trainium2 Bass kernel for nn_CrossAttention (dense_transformer).

Reference computation (per batch b, per stream s in {1,2}):
    q_s   = heads(x_s)                      # [H, N, D] slices of x_s
    kv_s  = x_s @ Wkv_s -> k_s, v_s         # [N, C] each
    gate_s= sigmoid(relu(x_s @ w1 + b1) @ w2 + b2)
    ctx_s = softmax_d( scale * k_s^T @ (v_s * gate_s) )   # [H, D, D]
    o_1   = q_1 @ ctx_2 ; o_2 = q_2 @ ctx_1  (cross)

Sharding: 8 cores = (stream s, batch b) pairs.  Core (s, b) projects
x_s[b] (kv + gate + ctx_s[b]) and then computes the OTHER stream's
output o_{1-s}[b] = q_{1-s}[b] @ softmax(ctx_s[b]).  No cross-core
communication; host concatenates outputs.

v2: host pre-transposes/pre-casts x (fp16), so the device does no
transposes and no DRAM spills.  All GEMMs fp16 (1 cycle/row); the gate
MLP can optionally run fp8e4 DoubleRow (0.5 cycles/row).  ctx is
accumulated in PSUM across all 32 n-chunks (two 8-head groups stacked
on partition halves -> one PSUM bank).
"""

import numpy as np
from contextlib import ExitStack

N = 4096
C = 1024
H = 16
D = 64
SCALE = D ** (-0.5)
NCH = N // 128       # 32 n-chunks of 128 rows

GATE_MODE = "fp16"   # 'fp16' | 'fp8'
S_X = 16.0           # fp8 activation scale for x
S_W = 256.0          # fp8 weight scale
S_H = 32.0           # fp8 scale for hidden h

_CACHE = {}


def _build_program(gate_mode, with_bias):
    import concourse.bass as bass
    import concourse.bacc as bacc
    import concourse.tile as tile
    import concourse.mybir as mybir

    F32 = mybir.dt.float32
    FP16 = mybir.dt.float16
    FP8 = mybir.dt.float8e4
    AF = mybir.ActivationFunctionType
    DR = mybir.MatmulPerfMode.DoubleRow
    fp8 = gate_mode == "fp8"
    HDT = FP8 if fp8 else FP16

    nc = bacc.Bacc("TRN2", target_bir_lowering=False, debug=False, num_devices=8)

    xt = nc.dram_tensor("xt", [C, N], FP16, kind="ExternalInput").ap()
    xqt = nc.dram_tensor("xqt", [C, N], FP16, kind="ExternalInput").ap()
    wkv = nc.dram_tensor("wkv", [C, 2 * C], FP16, kind="ExternalInput").ap()
    w1 = nc.dram_tensor("w1", [C, C], HDT, kind="ExternalInput").ap()
    w2 = nc.dram_tensor("w2", [C, C], HDT, kind="ExternalInput").ap()
    b1s = nc.dram_tensor("b1s", [128, 8], F32, kind="ExternalInput").ap()
    ident = nc.dram_tensor("ident", [128, 64], F32, kind="ExternalInput").ap()
    if fp8:
        xt8 = nc.dram_tensor("xt8", [C, N], FP8, kind="ExternalInput").ap()
    if with_bias:
        b2r = nc.dram_tensor("b2r", [1, C], FP16, kind="ExternalInput").ap()
    o = nc.dram_tensor("o", [N, C], F32, kind="ExternalOutput").ap()

    # activation post-scales to undo the fp8 pre-scales
    g1_scale = (S_H / (S_X * S_W)) if fp8 else 1.0
    g2_scale = (1.0 / (S_H * S_W)) if fp8 else 1.0
    ones_val = (S_H * S_W) if fp8 else 1.0

    with tile.TileContext(nc) as tc, ExitStack() as ctx:
        # ---------- persistent constants ----------
        cpool = ctx.enter_context(tc.tile_pool(name="consts", bufs=1))
        w1_sb = cpool.tile([128, 8, C], HDT, name="w1_sb")
        nc.sync.dma_start(w1_sb, w1.rearrange("(k p) m -> p k m", p=128))
        w2_sb = cpool.tile([128, 8, C], HDT, name="w2_sb")
        nc.sync.dma_start(w2_sb, w2.rearrange("(k p) m -> p k m", p=128))
        wkv_sb = cpool.tile([128, 8, 2 * C], FP16, name="wkv_sb")
        nc.sync.dma_start(wkv_sb, wkv.rearrange("(k p) m -> p k m", p=128))
        b1_sb = cpool.tile([128, 8], F32, name="b1_sb")
        nc.sync.dma_start(b1_sb, b1s)
        ident_sb = cpool.tile([128, 64], F32, name="ident_sb")
        nc.sync.dma_start(ident_sb, ident)
        if with_bias:
            ones_sb = cpool.tile([1, 128], F32, name="ones_sb")
            nc.vector.memset(ones_sb, ones_val)
            ones_r = cpool.tile([1, 128], FP16, name="ones_r")
            nc.vector.tensor_copy(ones_r, ones_sb)
            b2_r = cpool.tile([1, C], FP16, name="b2_r")
            nc.sync.dma_start(b2_r, b2r)

        spool = ctx.enter_context(tc.tile_pool(name="spairs", bufs=1))
        spairs = [spool.tile([128, 128], FP16, name=f"spair{j}") for j in range(8)]

        # ctx accumulator in PSUM: heads 0-7 on partitions 0-63, heads
        # 8-15 on 64-127; head h at cols (h%8)*64, layout [e, d].
        ctxps_pool = ctx.enter_context(
            tc.tile_pool(name="ctxps", bufs=1, space="PSUM")
        )
        ctx_ps = ctxps_pool.tile([128, 512], F32, name="ctx_ps")

        # phase-B xq tiles live alongside phase A so DMA prefetch overlaps
        bxq_pool = ctx.enter_context(tc.tile_pool(name="bxq", bufs=3))

        def emit_bxq_dma(blk):
            bx = bxq_pool.tile([128, 8, 512], FP16, name="bx", tag="bx")
            nc.sync.dma_start(
                bx,
                xqt.rearrange("(j p) n -> p j n", p=128)[
                    :, :, blk * 512:(blk + 1) * 512
                ],
            )
            return bx

        # =========================================================
        # Phase A: gates + kv projection + ctx accumulation, fused
        # =========================================================
        with ExitStack() as pa:
            xt_pool = pa.enter_context(tc.tile_pool(name="xt", bufs=2))
            if fp8:
                xt8_pool = pa.enter_context(tc.tile_pool(name="xt8", bufs=2))
            ht_pool = pa.enter_context(tc.tile_pool(name="ht", bufs=2))
            g_pool = pa.enter_context(tc.tile_pool(name="g", bufs=3))
            kf_pool = pa.enter_context(tc.tile_pool(name="kf", bufs=3))
            vg_pool = pa.enter_context(tc.tile_pool(name="vg", bufs=3))
            gps_pool = pa.enter_context(
                tc.tile_pool(name="gps", bufs=3, space="PSUM")
            )
            kvps_pool = pa.enter_context(
                tc.tile_pool(name="kvps", bufs=2, space="PSUM")
            )

            bx_tiles = {}
            pending = []  # (kf, vg, global_chunk) awaiting ctx matmuls

            def emit_ctx(kf_t, vg_t, gc):
                # start=True marks the whole 2KB PSUM bank (per partition)
                # as pending-zero, so issue it exactly once per partition
                # half; the other heads' first writes then init via the
                # pending-zero overwrite instead of accumulating garbage.
                for h in range(H):
                    nc.tensor.matmul(
                        ctx_ps[
                            (h // 8) * 64:(h // 8) * 64 + 64,
                            (h % 8) * 64:(h % 8) * 64 + 64,
                        ],
                        vg_t[:, h * D:(h + 1) * D],
                        kf_t[:, h * D:(h + 1) * D],
                        start=(gc == 0 and h % 8 == 0),
                        stop=(gc == NCH - 1),
                        skip_group_check=True,
                    )

            for blk in range(4):
                xt_in = xt_pool.tile([128, 8, C], FP16, name="xt_in", tag="xt")
                nc.sync.dma_start(
                    xt_in,
                    xt.rearrange("(k p) n -> p k n", p=128)[
                        :, :, blk * 1024:(blk + 1) * 1024
                    ],
                )
                if fp8:
                    xt8_in = xt8_pool.tile([128, 8, C], FP8, name="xt8_in", tag="xt8")
                    nc.sync.dma_start(
                        xt8_in,
                        xt8.rearrange("(k p) n -> p k n", p=128)[
                            :, :, blk * 1024:(blk + 1) * 1024
                        ],
                    )
                if blk < 3:
                    bx_tiles[blk] = emit_bxq_dma(blk)

                # ---- gate1: hT[m-tile, n] = relu(x@w1+b1).T ----
                ht = ht_pool.tile([128, 8, C], HDT, name="ht", tag="ht")
                for m in range(8):
                    pss = [
                        gps_pool.tile([128, 512], F32, name="g1ps", tag="gps")
                        for _ in range(2)
                    ]
                    if fp8:
                        for kp in range(4):
                            lhs = w1_sb[:, 2 * kp:2 * kp + 2, m * 128:(m + 1) * 128]
                            for half in range(2):
                                nc.tensor.matmul(
                                    pss[half],
                                    lhs,
                                    xt8_in[:, 2 * kp:2 * kp + 2,
                                           half * 512:(half + 1) * 512],
                                    start=(kp == 0),
                                    stop=(kp == 3),
                                    perf_mode=DR,
                                )
                    else:
                        for k in range(8):
                            lhs = w1_sb[:, k, m * 128:(m + 1) * 128]
                            for half in range(2):
                                nc.tensor.matmul(
                                    pss[half],
                                    lhs,
                                    xt_in[:, k, half * 512:(half + 1) * 512],
                                    start=(k == 0),
                                    stop=(k == 7),
                                )
                    for half in range(2):
                        nc.scalar.activation(
                            ht[:, m, half * 512:(half + 1) * 512],
                            pss[half],
                            AF.Relu,
                            bias=b1_sb[:, m:m + 1],
                            scale=g1_scale,
                        )

                # ---- per chunk: gate2 -> kv -> (delayed) ctx ----
                for c in range(8):
                    gc = blk * 8 + c
                    gt = g_pool.tile([128, C], FP16, name="gt", tag="gt")
                    for t in range(2):
                        ps2 = gps_pool.tile([128, 512], F32, name="g2ps", tag="gps")
                        if fp8:
                            for kp in range(4):
                                nc.tensor.matmul(
                                    ps2,
                                    ht[:, 2 * kp:2 * kp + 2, c * 128:(c + 1) * 128],
                                    w2_sb[:, 2 * kp:2 * kp + 2,
                                          t * 512:(t + 1) * 512],
                                    start=(kp == 0),
                                    stop=(kp == 3 and not with_bias),
                                    perf_mode=DR,
                                )
                        else:
                            for k in range(8):
                                nc.tensor.matmul(
                                    ps2,
                                    ht[:, k, c * 128:(c + 1) * 128],
                                    w2_sb[:, k, t * 512:(t + 1) * 512],
                                    start=(k == 0),
                                    stop=(k == 7 and not with_bias),
                                )
                        if with_bias:
                            nc.tensor.matmul(
                                ps2,
                                ones_r,
                                b2_r[:, t * 512:(t + 1) * 512],
                                start=False,
                                stop=True,
                            )
                        nc.scalar.activation(
                            gt[:, t * 512:(t + 1) * 512], ps2, AF.Sigmoid,
                            scale=g2_scale,
                        )

                    # kv projection for this chunk; k and v psum halves
                    ps_k = kvps_pool.tile([128, C], F32, name="ps_k", tag="kvps")
                    ps_v = kvps_pool.tile([128, C], F32, name="ps_v", tag="kvps")
                    for k in range(8):
                        lhs = xt_in[:, k, c * 128:(c + 1) * 128]
                        for t in range(2):
                            nc.tensor.matmul(
                                ps_k[:, t * 512:(t + 1) * 512],
                                lhs,
                                wkv_sb[:, k, t * 512:(t + 1) * 512],
                                start=(k == 0),
                                stop=(k == 7),
                            )
                        for t in range(2):
                            nc.tensor.matmul(
                                ps_v[:, t * 512:(t + 1) * 512],
                                lhs,
                                wkv_sb[:, k, C + t * 512:C + (t + 1) * 512],
                                start=(k == 0),
                                stop=(k == 7),
                            )
                    kf = kf_pool.tile([128, C], FP16, name="kf", tag="kf")
                    nc.scalar.copy(kf, ps_k)
                    vg = vg_pool.tile([128, C], FP16, name="vg", tag="vg")
                    nc.vector.tensor_mul(vg, ps_v, gt)

                    # ctx for the PREVIOUS chunk (kf/vg conversions for it
                    # ran while this chunk's kv matmuls streamed)
                    if pending:
                        emit_ctx(*pending.pop(0))
                    pending.append((kf, vg, gc))

            while pending:
                emit_ctx(*pending.pop(0))

        # =========================================================
        # Softmax over d (free dim of ctxT) + block-diag S pairs
        # =========================================================
        with ExitStack() as sm:
            smp = sm.enter_context(tc.tile_pool(name="smpool", bufs=1))
            smps = sm.enter_context(tc.tile_pool(name="smps", bufs=2, space="PSUM"))
            maxs = smp.tile([128, 8], F32, name="maxs")
            nc.vector.tensor_reduce(
                maxs,
                ctx_ps.rearrange("p (b d) -> p b d", b=8),
                axis=mybir.AxisListType.X,
                op=mybir.AluOpType.max,
            )
            cmx = smp.tile([128, 512], F32, name="cmx")
            nc.vector.tensor_sub(
                cmx.rearrange("p (h d) -> p h d", h=8),
                ctx_ps.rearrange("p (h d) -> p h d", h=8),
                maxs.unsqueeze(-1).broadcast_to([128, 8, 64]),
            )
            et = smp.tile([128, 512], F32, name="et")
            nc.scalar.activation(et, cmx, AF.Exp, scale=float(SCALE))
            sums = smp.tile([128, 8], F32, name="sums")
            nc.vector.tensor_reduce(
                sums,
                et.rearrange("p (b d) -> p b d", b=8),
                axis=mybir.AxisListType.X,
                op=mybir.AluOpType.add,
            )
            recs = smp.tile([128, 8], F32, name="recs")
            nc.vector.reciprocal(recs, sums)
            st = smp.tile([128, 512], F32, name="st")
            nc.vector.tensor_mul(
                st.rearrange("p (h d) -> p h d", h=8),
                et.rearrange("p (h d) -> p h d", h=8),
                recs.unsqueeze(-1).broadcast_to([128, 8, 64]),
            )
            # st rows e (64 per half), cols d per head.  Transposing the
            # side-by-side pair [ctxT_2j | ctxT_2j+1] ([64, 128]) gives
            # [S_2j stacked above S_2j+1] ([128, 64]); scatter block-diag.
            zero_sb = smp.tile([128, 128], FP16, name="zero_sb")
            nc.vector.memset(zero_sb, 0.0)
            for j in range(8):
                half = j // 4  # heads 0-7 in lower partitions, 8-15 upper
                base = half * 64
                colj = (2 * j) % 8
                tp = smps.tile([128, 64], F32, name="smtp", tag="smtp")
                nc.tensor.transpose(
                    tp,
                    st[base:base + 64, colj * 64:(colj + 2) * 64],
                    ident_sb[base:base + 64, :],
                )
                nc.vector.tensor_copy(spairs[j], zero_sb)
                nc.vector.tensor_copy(spairs[j][0:64, 0:64], tp[0:64, :])
                nc.vector.tensor_copy(spairs[j][64:128, 64:128], tp[64:128, :])

        # =========================================================
        # Phase B: o[nchunk, j*128:(j+1)*128] = q_pair @ blockdiag(S)
        # =========================================================
        with ExitStack() as pb:
            oo_pool = pb.enter_context(tc.tile_pool(name="bo", bufs=6))
            bops_pool = pb.enter_context(
                tc.tile_pool(name="bops", bufs=4, space="PSUM")
            )
            for blk in range(8):
                if blk + 3 < 8:
                    bx_tiles[blk + 3] = emit_bxq_dma(blk + 3)
                bx = bx_tiles.pop(blk)
                for c4 in range(4):
                    oo = oo_pool.tile([128, C], F32, name="oo", tag="oo")
                    for half in range(2):
                        ops = bops_pool.tile([128, 512], F32, name="ops", tag="ops")
                        for jj in range(4):
                            j = half * 4 + jj
                            nc.tensor.matmul(
                                ops[:, jj * 128:(jj + 1) * 128],
                                bx[:, j, c4 * 128:(c4 + 1) * 128],
                                spairs[j],
                                start=True,
                                stop=True,
                                skip_group_check=True,
                            )
                        if half == 0:
                            nc.vector.tensor_copy(
                                oo[:, half * 512:(half + 1) * 512], ops
                            )
                        else:
                            nc.scalar.copy(
                                oo[:, half * 512:(half + 1) * 512], ops
                            )
                    nch = blk * 4 + c4
                    nc.sync.dma_start(o[nch * 128:(nch + 1) * 128, :], oo)

    nc.compile()
    return nc


def _get_program(gate_mode=None, with_bias=False):
    if gate_mode is None:
        gate_mode = GATE_MODE
    key = (gate_mode, bool(with_bias))
    if key not in _CACHE:
        _CACHE[key] = _build_program(gate_mode, with_bias)
    return _CACHE[key]


def make_in_maps(x1, x2, Wkv1, Wkv2, g1_w1, g1_b1, g1_w2, g1_b2,
                 g2_w1, g2_b1, g2_w2, g2_b2, gate_mode=None):
    """Core (s, b): cores 0-3 = (s=0, b), cores 4-7 = (s=1, b)."""
    import ml_dtypes
    if gate_mode is None:
        gate_mode = GATE_MODE
    fp8 = gate_mode == "fp8"
    F8 = ml_dtypes.float8_e4m3
    ident = np.vstack([np.eye(64, dtype=np.float32)] * 2)

    def prep_stream(x, wkv, w1, b1, w2, b2):
        m = {
            "xt": x.T.astype(np.float16, order="C"),
            "wkv": wkv.astype(np.float16),
            "ident": ident,
        }
        if fp8:
            m["xt8"] = (x.T * S_X).astype(F8, order="C")
            m["w1"] = (w1 * S_W).astype(F8)
            m["w2"] = (w2 * S_W).astype(F8)
            m["b1s"] = np.ascontiguousarray((S_H * b1).reshape(8, 128).T)
        else:
            m["w1"] = w1.astype(np.float16)
            m["w2"] = w2.astype(np.float16)
            m["b1s"] = np.ascontiguousarray(b1.reshape(8, 128).T)
        m["b2r"] = b2.reshape(1, C).astype(np.float16)
        return m

    in_maps = []
    for core in range(8):
        s, b = core // 4, core % 4
        if s == 0:
            m = prep_stream(x1[b], Wkv1, g1_w1, g1_b1, g1_w2, g1_b2)
            m["xqt"] = x2[b].T.astype(np.float16, order="C")
        else:
            m = prep_stream(x2[b], Wkv2, g2_w1, g2_b1, g2_w2, g2_b2)
            m["xqt"] = x1[b].T.astype(np.float16, order="C")
        in_maps.append(m)
    return in_maps


def kernel(x1, x2, Wkv1, Wkv2, g1_w1, g1_b1, g1_w2, g1_b2,
           g2_w1, g2_b1, g2_w2, g2_b2, _runner=None):
    """Full-input entry point.  Returns (o1, o2), each [4, 4096, 1024] f32."""
    from concourse.bass_utils import run_bass_kernel_spmd

    args = [np.asarray(a, dtype=np.float32) for a in
            (x1, x2, Wkv1, Wkv2, g1_w1, g1_b1, g1_w2, g1_b2,
             g2_w1, g2_b1, g2_w2, g2_b2)]
    with_bias = bool(np.any(args[7]) or np.any(args[11]))  # g1_b2, g2_b2
    nc = _get_program(GATE_MODE, with_bias)
    in_maps = make_in_maps(*args)
    if not with_bias:
        for m in in_maps:
            m.pop("b2r", None)
    if _runner is None:
        res = run_bass_kernel_spmd(nc, in_maps, core_ids=list(range(8)))
        results = res.results
    else:
        results = _runner(nc, in_maps)

    B = x1.shape[0]
    o1 = np.empty((B, N, C), dtype=np.float32)
    o2 = np.empty((B, N, C), dtype=np.float32)
    for core in range(8):
        s, b = core // 4, core % 4
        out = results[core]["o"]
        if s == 0:
            o2[b] = out   # core projected x1 -> ctx1 -> o2 = q2 @ ctx1
        else:
            o1[b] = out
    return (o1, o2)


# revision 13
# speedup vs baseline: 1.4284x; 1.1266x over previous
"""Trainium2 Bass kernel for nn_CrossAttention (dense_transformer).

Reference computation (per batch b, per stream s in {1,2}):
    q_s   = heads(x_s)                      # [H, N, D] slices of x_s
    kv_s  = x_s @ Wkv_s -> k_s, v_s         # [N, C] each
    gate_s= sigmoid(relu(x_s @ w1 + b1) @ w2 + b2)
    ctx_s = softmax_d( scale * k_s^T @ (v_s * gate_s) )   # [H, D, D]
    o_1   = q_1 @ ctx_2 ; o_2 = q_2 @ ctx_1  (cross)

Sharding: 8 cores = (stream s, batch b) pairs.  Core (s, b) projects
x_s[b] (kv + gate + ctx_s[b]) and then computes the OTHER stream's
output o_{1-s}[b] = q_{1-s}[b] @ softmax(ctx_s[b]).  No cross-core
communication; host concatenates outputs.

v2: host pre-transposes/pre-casts x (fp16), so the device does no
transposes and no DRAM spills.  All GEMMs fp16 (1 cycle/row); the gate
MLP can optionally run fp8e4 DoubleRow (0.5 cycles/row).  ctx is
accumulated in PSUM across all 32 n-chunks (two 8-head groups stacked
on partition halves -> one PSUM bank).
"""

import numpy as np
from contextlib import ExitStack

N = 4096
C = 1024
H = 16
D = 64
SCALE = D ** (-0.5)
NCH = N // 128       # 32 n-chunks of 128 rows

GATE_MODE = "fp8"    # 'fp16' | 'fp8'
S_X = 16.0           # fp8 activation scale for x
S_W = 256.0          # fp8 weight scale
S_H = 32.0           # fp8 scale for hidden h

_CACHE = {}


def _build_program(gate_mode, with_bias):
    import concourse.bass as bass
    import concourse.bacc as bacc
    import concourse.tile as tile
    import concourse.mybir as mybir

    F32 = mybir.dt.float32
    FP16 = mybir.dt.float16
    FP8 = mybir.dt.float8e4
    AF = mybir.ActivationFunctionType
    DR = mybir.MatmulPerfMode.DoubleRow
    fp8 = gate_mode == "fp8"
    HDT = FP8 if fp8 else FP16

    nc = bacc.Bacc("TRN2", target_bir_lowering=False, debug=False, num_devices=8)

    xt = nc.dram_tensor("xt", [C, N], FP16, kind="ExternalInput").ap()
    xqt = nc.dram_tensor("xqt", [C, N], FP16, kind="ExternalInput").ap()
    wkv = nc.dram_tensor("wkv", [C, 2 * C], FP16, kind="ExternalInput").ap()
    w1 = nc.dram_tensor("w1", [C, C], HDT, kind="ExternalInput").ap()
    w2 = nc.dram_tensor("w2", [C, C], HDT, kind="ExternalInput").ap()
    b1s = nc.dram_tensor("b1s", [128, 8], F32, kind="ExternalInput").ap()
    ident = nc.dram_tensor("ident", [128, 64], F32, kind="ExternalInput").ap()
    if fp8:
        xt8 = nc.dram_tensor("xt8", [C, N], FP8, kind="ExternalInput").ap()
    if with_bias:
        b2r = nc.dram_tensor("b2r", [1, C], FP16, kind="ExternalInput").ap()
    o = nc.dram_tensor("o", [N, C], FP16, kind="ExternalOutput").ap()

    # activation post-scales to undo the fp8 pre-scales
    g1_scale = (S_H / (S_X * S_W)) if fp8 else 1.0
    g2_scale = (1.0 / (S_H * S_W)) if fp8 else 1.0
    ones_val = (S_H * S_W) if fp8 else 1.0

    with tile.TileContext(nc) as tc, ExitStack() as ctx:
        # ---------- persistent constants ----------
        cpool = ctx.enter_context(tc.tile_pool(name="consts", bufs=1))
        w1_sb = cpool.tile([128, 8, C], HDT, name="w1_sb")
        nc.sync.dma_start(w1_sb, w1.rearrange("(k p) m -> p k m", p=128))
        w2_sb = cpool.tile([128, 8, C], HDT, name="w2_sb")
        nc.sync.dma_start(w2_sb, w2.rearrange("(k p) m -> p k m", p=128))
        wkv_sb = cpool.tile([128, 8, 2 * C], FP16, name="wkv_sb")
        nc.sync.dma_start(wkv_sb, wkv.rearrange("(k p) m -> p k m", p=128))
        b1_sb = cpool.tile([128, 8], F32, name="b1_sb")
        nc.sync.dma_start(b1_sb, b1s)
        ident_sb = cpool.tile([128, 64], F32, name="ident_sb")
        nc.sync.dma_start(ident_sb, ident)
        if with_bias:
            ones_sb = cpool.tile([1, 128], F32, name="ones_sb")
            nc.vector.memset(ones_sb, ones_val)
            ones_r = cpool.tile([1, 128], FP16, name="ones_r")
            nc.vector.tensor_copy(ones_r, ones_sb)
            b2_r = cpool.tile([1, C], FP16, name="b2_r")
            nc.sync.dma_start(b2_r, b2r)

        spool = ctx.enter_context(tc.tile_pool(name="spairs", bufs=1))
        spairs = [spool.tile([128, 128], FP16, name=f"spair{j}") for j in range(8)]

        # ctx accumulator in PSUM: heads 0-7 on partitions 0-63, heads
        # 8-15 on 64-127; head h at cols (h%8)*64, layout [e, d].
        ctxps_pool = ctx.enter_context(
            tc.tile_pool(name="ctxps", bufs=1, space="PSUM")
        )
        ctx_ps = ctxps_pool.tile([128, 512], F32, name="ctx_ps")

        # phase-B xq tiles live alongside phase A so DMA prefetch overlaps
        bxq_pool = ctx.enter_context(tc.tile_pool(name="bxq", bufs=6))

        def emit_bxq_dma(blk):
            bx = bxq_pool.tile([128, 8, 512], FP16, name="bx", tag="bx")
            nc.sync.dma_start(
                bx,
                xqt.rearrange("(j p) n -> p j n", p=128)[
                    :, :, blk * 512:(blk + 1) * 512
                ],
            )
            return bx

        # =========================================================
        # Phase A: gates + kv projection + ctx accumulation, fused
        # =========================================================
        with ExitStack() as pa:
            xt_pool = pa.enter_context(tc.tile_pool(name="xt", bufs=2))
            if fp8:
                xt8_pool = pa.enter_context(tc.tile_pool(name="xt8", bufs=2))
            ht_pool = pa.enter_context(tc.tile_pool(name="ht", bufs=2))
            g_pool = pa.enter_context(tc.tile_pool(name="g", bufs=3))
            kf_pool = pa.enter_context(tc.tile_pool(name="kf", bufs=3))
            vg_pool = pa.enter_context(tc.tile_pool(name="vg", bufs=3))
            gps_pool = pa.enter_context(
                tc.tile_pool(name="gps", bufs=3, space="PSUM")
            )
            kvps_pool = pa.enter_context(
                tc.tile_pool(name="kvps", bufs=2, space="PSUM")
            )

            bx_tiles = {}
            pending = []  # (kf, vg, global_chunk) awaiting ctx matmuls

            def emit_ctx(kf_t, vg_t, gc):
                # start=True marks the whole 2KB PSUM bank (per partition)
                # as pending-zero, so issue it exactly once per partition
                # half; the other heads' first writes then init via the
                # pending-zero overwrite instead of accumulating garbage.
                for h in range(H):
                    nc.tensor.matmul(
                        ctx_ps[
                            (h // 8) * 64:(h // 8) * 64 + 64,
                            (h % 8) * 64:(h % 8) * 64 + 64,
                        ],
                        vg_t[:, h * D:(h + 1) * D],
                        kf_t[:, h * D:(h + 1) * D],
                        start=(gc == 0 and h % 8 == 0),
                        stop=(gc == NCH - 1),
                        skip_group_check=True,
                    )

            for blk in range(4):
                xt_in = xt_pool.tile([128, 8, C], FP16, name="xt_in", tag="xt")
                nc.sync.dma_start(
                    xt_in,
                    xt.rearrange("(k p) n -> p k n", p=128)[
                        :, :, blk * 1024:(blk + 1) * 1024
                    ],
                )
                if fp8:
                    xt8_in = xt8_pool.tile([128, 8, C], FP8, name="xt8_in", tag="xt8")
                    nc.sync.dma_start(
                        xt8_in,
                        xt8.rearrange("(k p) n -> p k n", p=128)[
                            :, :, blk * 1024:(blk + 1) * 1024
                        ],
                    )
                # prefetch phase-B xq tiles while DMA is quiet (2 per block)
                for pf in (2 * blk, 2 * blk + 1):
                    if pf < 6:
                        bx_tiles[pf] = emit_bxq_dma(pf)

                # ---- gate1: hT[m-tile, n] = relu(x@w1+b1).T ----
                ht = ht_pool.tile([128, 8, C], HDT, name="ht", tag="ht")
                for m in range(8):
                    pss = [
                        gps_pool.tile([128, 512], F32, name="g1ps", tag="gps")
                        for _ in range(2)
                    ]
                    if fp8:
                        for kp in range(4):
                            lhs = w1_sb[:, 2 * kp:2 * kp + 2, m * 128:(m + 1) * 128]
                            for half in range(2):
                                nc.tensor.matmul(
                                    pss[half],
                                    lhs,
                                    xt8_in[:, 2 * kp:2 * kp + 2,
                                           half * 512:(half + 1) * 512],
                                    start=(kp == 0),
                                    stop=(kp == 3),
                                    perf_mode=DR,
                                )
                    else:
                        for k in range(8):
                            lhs = w1_sb[:, k, m * 128:(m + 1) * 128]
                            for half in range(2):
                                nc.tensor.matmul(
                                    pss[half],
                                    lhs,
                                    xt_in[:, k, half * 512:(half + 1) * 512],
                                    start=(k == 0),
                                    stop=(k == 7),
                                )
                    for half in range(2):
                        nc.scalar.activation(
                            ht[:, m, half * 512:(half + 1) * 512],
                            pss[half],
                            AF.Relu,
                            bias=b1_sb[:, m:m + 1],
                            scale=g1_scale,
                        )

                # ---- per chunk: gate2 -> kv -> (delayed) ctx ----
                for c in range(8):
                    gc = blk * 8 + c
                    gt = g_pool.tile([128, C], FP16, name="gt", tag="gt")
                    for t in range(2):
                        ps2 = gps_pool.tile([128, 512], F32, name="g2ps", tag="gps")
                        if fp8:
                            for kp in range(4):
                                nc.tensor.matmul(
                                    ps2,
                                    ht[:, 2 * kp:2 * kp + 2, c * 128:(c + 1) * 128],
                                    w2_sb[:, 2 * kp:2 * kp + 2,
                                          t * 512:(t + 1) * 512],
                                    start=(kp == 0),
                                    stop=(kp == 3 and not with_bias),
                                    perf_mode=DR,
                                )
                        else:
                            for k in range(8):
                                nc.tensor.matmul(
                                    ps2,
                                    ht[:, k, c * 128:(c + 1) * 128],
                                    w2_sb[:, k, t * 512:(t + 1) * 512],
                                    start=(k == 0),
                                    stop=(k == 7 and not with_bias),
                                )
                        if with_bias:
                            nc.tensor.matmul(
                                ps2,
                                ones_r,
                                b2_r[:, t * 512:(t + 1) * 512],
                                start=False,
                                stop=True,
                            )
                        nc.scalar.activation(
                            gt[:, t * 512:(t + 1) * 512], ps2, AF.Sigmoid,
                            scale=g2_scale,
                        )

                    # kv projection for this chunk; k and v psum halves
                    ps_k = kvps_pool.tile([128, C], F32, name="ps_k", tag="kvps")
                    ps_v = kvps_pool.tile([128, C], F32, name="ps_v", tag="kvps")
                    for k in range(8):
                        lhs = xt_in[:, k, c * 128:(c + 1) * 128]
                        for t in range(2):
                            nc.tensor.matmul(
                                ps_k[:, t * 512:(t + 1) * 512],
                                lhs,
                                wkv_sb[:, k, t * 512:(t + 1) * 512],
                                start=(k == 0),
                                stop=(k == 7),
                            )
                        for t in range(2):
                            nc.tensor.matmul(
                                ps_v[:, t * 512:(t + 1) * 512],
                                lhs,
                                wkv_sb[:, k, C + t * 512:C + (t + 1) * 512],
                                start=(k == 0),
                                stop=(k == 7),
                            )
                    kf = kf_pool.tile([128, C], FP16, name="kf", tag="kf")
                    nc.scalar.copy(kf, ps_k)
                    vg = vg_pool.tile([128, C], FP16, name="vg", tag="vg")
                    nc.vector.tensor_mul(vg, ps_v, gt)

                    # ctx for the PREVIOUS chunk (kf/vg conversions for it
                    # ran while this chunk's kv matmuls streamed)
                    if pending:
                        emit_ctx(*pending.pop(0))
                    pending.append((kf, vg, gc))

            while pending:
                emit_ctx(*pending.pop(0))

        # =========================================================
        # Softmax over d (free dim of ctxT) + block-diag S pairs
        # =========================================================
        with ExitStack() as sm:
            smp = sm.enter_context(tc.tile_pool(name="smpool", bufs=1))
            smps = sm.enter_context(tc.tile_pool(name="smps", bufs=2, space="PSUM"))
            maxs = smp.tile([128, 8], F32, name="maxs")
            nc.vector.tensor_reduce(
                maxs,
                ctx_ps.rearrange("p (b d) -> p b d", b=8),
                axis=mybir.AxisListType.X,
                op=mybir.AluOpType.max,
            )
            cmx = smp.tile([128, 512], F32, name="cmx")
            nc.vector.tensor_sub(
                cmx.rearrange("p (h d) -> p h d", h=8),
                ctx_ps.rearrange("p (h d) -> p h d", h=8),
                maxs.unsqueeze(-1).broadcast_to([128, 8, 64]),
            )
            et = smp.tile([128, 512], F32, name="et")
            nc.scalar.activation(et, cmx, AF.Exp, scale=float(SCALE))
            sums = smp.tile([128, 8], F32, name="sums")
            nc.vector.tensor_reduce(
                sums,
                et.rearrange("p (b d) -> p b d", b=8),
                axis=mybir.AxisListType.X,
                op=mybir.AluOpType.add,
            )
            recs = smp.tile([128, 8], F32, name="recs")
            nc.vector.reciprocal(recs, sums)
            st = smp.tile([128, 512], F32, name="st")
            nc.vector.tensor_mul(
                st.rearrange("p (h d) -> p h d", h=8),
                et.rearrange("p (h d) -> p h d", h=8),
                recs.unsqueeze(-1).broadcast_to([128, 8, 64]),
            )
            # st rows e (64 per half), cols d per head.  Transposing the
            # side-by-side pair [ctxT_2j | ctxT_2j+1] ([64, 128]) gives
            # [S_2j stacked above S_2j+1] ([128, 64]); scatter block-diag.
            zero_sb = smp.tile([128, 128], FP16, name="zero_sb")
            nc.vector.memset(zero_sb, 0.0)
            for j in range(8):
                half = j // 4  # heads 0-7 in lower partitions, 8-15 upper
                base = half * 64
                colj = (2 * j) % 8
                tp = smps.tile([128, 64], F32, name="smtp", tag="smtp")
                nc.tensor.transpose(
                    tp,
                    st[base:base + 64, colj * 64:(colj + 2) * 64],
                    ident_sb[base:base + 64, :],
                )
                nc.vector.tensor_copy(spairs[j], zero_sb)
                nc.vector.tensor_copy(spairs[j][0:64, 0:64], tp[0:64, :])
                nc.vector.tensor_copy(spairs[j][64:128, 64:128], tp[64:128, :])

        # =========================================================
        # Phase B: o[nchunk, j*128:(j+1)*128] = q_pair @ blockdiag(S)
        # =========================================================
        with ExitStack() as pb:
            oo_pool = pb.enter_context(tc.tile_pool(name="bo", bufs=6))
            bops_pool = pb.enter_context(
                tc.tile_pool(name="bops", bufs=4, space="PSUM")
            )
            for blk in range(8):
                if blk + 6 < 8:
                    bx_tiles[blk + 6] = emit_bxq_dma(blk + 6)
                bx = bx_tiles.pop(blk)
                for c4 in range(4):
                    oo = oo_pool.tile([128, C], FP16, name="oo", tag="oo")
                    for half in range(2):
                        ops = bops_pool.tile([128, 512], F32, name="ops", tag="ops")
                        for jj in range(4):
                            j = half * 4 + jj
                            nc.tensor.matmul(
                                ops[:, jj * 128:(jj + 1) * 128],
                                bx[:, j, c4 * 128:(c4 + 1) * 128],
                                spairs[j],
                                start=True,
                                stop=True,
                                skip_group_check=True,
                            )
                        if half == 0:
                            nc.vector.tensor_copy(
                                oo[:, half * 512:(half + 1) * 512], ops
                            )
                        else:
                            nc.scalar.copy(
                                oo[:, half * 512:(half + 1) * 512], ops
                            )
                    nch = blk * 4 + c4
                    nc.sync.dma_start(o[nch * 128:(nch + 1) * 128, :], oo)

    nc.compile()
    return nc


def _get_program(gate_mode=None, with_bias=False):
    if gate_mode is None:
        gate_mode = GATE_MODE
    key = (gate_mode, bool(with_bias))
    if key not in _CACHE:
        _CACHE[key] = _build_program(gate_mode, with_bias)
    return _CACHE[key]


def make_in_maps(x1, x2, Wkv1, Wkv2, g1_w1, g1_b1, g1_w2, g1_b2,
                 g2_w1, g2_b1, g2_w2, g2_b2, gate_mode=None):
    """Core (s, b): cores 0-3 = (s=0, b), cores 4-7 = (s=1, b)."""
    import ml_dtypes
    if gate_mode is None:
        gate_mode = GATE_MODE
    fp8 = gate_mode == "fp8"
    F8 = ml_dtypes.float8_e4m3
    ident = np.vstack([np.eye(64, dtype=np.float32)] * 2)

    def prep_stream(x, wkv, w1, b1, w2, b2):
        m = {
            "xt": x.T.astype(np.float16, order="C"),
            "wkv": wkv.astype(np.float16),
            "ident": ident,
        }
        if fp8:
            m["xt8"] = (x.T * S_X).astype(F8, order="C")
            m["w1"] = (w1 * S_W).astype(F8)
            m["w2"] = (w2 * S_W).astype(F8)
            m["b1s"] = np.ascontiguousarray((S_H * b1).reshape(8, 128).T)
        else:
            m["w1"] = w1.astype(np.float16)
            m["w2"] = w2.astype(np.float16)
            m["b1s"] = np.ascontiguousarray(b1.reshape(8, 128).T)
        m["b2r"] = b2.reshape(1, C).astype(np.float16)
        return m

    in_maps = []
    for core in range(8):
        s, b = core // 4, core % 4
        if s == 0:
            m = prep_stream(x1[b], Wkv1, g1_w1, g1_b1, g1_w2, g1_b2)
            m["xqt"] = x2[b].T.astype(np.float16, order="C")
        else:
            m = prep_stream(x2[b], Wkv2, g2_w1, g2_b1, g2_w2, g2_b2)
            m["xqt"] = x1[b].T.astype(np.float16, order="C")
        in_maps.append(m)
    return in_maps


def kernel(x1, x2, Wkv1, Wkv2, g1_w1, g1_b1, g1_w2, g1_b2,
           g2_w1, g2_b1, g2_w2, g2_b2, _runner=None):
    """Full-input entry point.  Returns (o1, o2), each [4, 4096, 1024] f32."""
    from concourse.bass_utils import run_bass_kernel_spmd

    args = [np.asarray(a, dtype=np.float32) for a in
            (x1, x2, Wkv1, Wkv2, g1_w1, g1_b1, g1_w2, g1_b2,
             g2_w1, g2_b1, g2_w2, g2_b2)]
    with_bias = bool(np.any(args[7]) or np.any(args[11]))  # g1_b2, g2_b2
    nc = _get_program(GATE_MODE, with_bias)
    in_maps = make_in_maps(*args)
    if not with_bias:
        for m in in_maps:
            m.pop("b2r", None)
    if _runner is None:
        res = run_bass_kernel_spmd(nc, in_maps, core_ids=list(range(8)))
        results = res.results
    else:
        results = _runner(nc, in_maps)

    B = x1.shape[0]
    o1 = np.empty((B, N, C), dtype=np.float32)
    o2 = np.empty((B, N, C), dtype=np.float32)
    for core in range(8):
        s, b = core // 4, core % 4
        out = np.asarray(results[core]["o"], dtype=np.float32)
        if s == 0:
            o2[b] = out   # core projected x1 -> ctx1 -> o2 = q2 @ ctx1
        else:
            o1[b] = out
    return (o1, o2)


# revision 19
# speedup vs baseline: 1.6820x; 1.1775x over previous
"""Trainium2 Bass kernel for nn_CrossAttention (dense_transformer).

Reference computation (per batch b, per stream s in {1,2}):
    q_s   = heads(x_s)                      # [H, N, D] slices of x_s
    kv_s  = x_s @ Wkv_s -> k_s, v_s         # [N, C] each
    gate_s= sigmoid(relu(x_s @ w1 + b1) @ w2 + b2)
    ctx_s = softmax_d( scale * k_s^T @ (v_s * gate_s) )   # [H, D, D]
    o_1   = q_1 @ ctx_2 ; o_2 = q_2 @ ctx_1  (cross)

Sharding: 8 cores = (stream s, batch b) pairs.  Core (s, b) projects
x_s[b] (kv + gate + ctx_s[b]) and then computes the OTHER stream's
output o_{1-s}[b] = q_{1-s}[b] @ softmax(ctx_s[b]).  No cross-core
communication; host concatenates outputs.

v2: host pre-transposes/pre-casts x (fp16), so the device does no
transposes and no DRAM spills.  All GEMMs fp16 (1 cycle/row); the gate
MLP can optionally run fp8e4 DoubleRow (0.5 cycles/row).  ctx is
accumulated in PSUM across all 32 n-chunks (two 8-head groups stacked
on partition halves -> one PSUM bank).
"""

import numpy as np
from contextlib import ExitStack

N = 4096
C = 1024
H = 16
D = 64
SCALE = D ** (-0.5)
NCH = N // 128       # 32 n-chunks of 128 rows

GATE_MODE = "fp8"    # 'fp16' | 'fp8'
S_X = 16.0           # fp8 activation scale for x
S_W = 256.0          # fp8 weight scale
S_H = 32.0           # fp8 scale for hidden h

_CACHE = {}


def _build_program(gate_mode, with_bias):
    import concourse.bass as bass
    import concourse.bacc as bacc
    import concourse.tile as tile
    import concourse.mybir as mybir

    F32 = mybir.dt.float32
    FP16 = mybir.dt.float16
    FP8 = mybir.dt.float8e4
    AF = mybir.ActivationFunctionType
    DR = mybir.MatmulPerfMode.DoubleRow
    fp8 = gate_mode == "fp8"
    HDT = FP8 if fp8 else FP16

    nc = bacc.Bacc("TRN2", target_bir_lowering=False, debug=False, num_devices=8)

    xt = nc.dram_tensor("xt", [C, N], FP16, kind="ExternalInput").ap()
    xqt = nc.dram_tensor("xqt", [C, N], FP16, kind="ExternalInput").ap()
    wkv = nc.dram_tensor("wkv", [C, 2 * C], FP16, kind="ExternalInput").ap()
    w1 = nc.dram_tensor("w1", [C, C], HDT, kind="ExternalInput").ap()
    w2 = nc.dram_tensor("w2", [C, C], HDT, kind="ExternalInput").ap()
    b1s = nc.dram_tensor("b1s", [128, 8], F32, kind="ExternalInput").ap()
    ident = nc.dram_tensor("ident", [128, 64], F32, kind="ExternalInput").ap()
    if fp8:
        xt8 = nc.dram_tensor("xt8", [C, N], FP8, kind="ExternalInput").ap()
    if with_bias:
        b2r = nc.dram_tensor("b2r", [1, C], FP16, kind="ExternalInput").ap()
    o = nc.dram_tensor("o", [N, C], FP16, kind="ExternalOutput").ap()

    # activation post-scales to undo the fp8 pre-scales
    g1_scale = (S_H / (S_X * S_W)) if fp8 else 1.0
    g2_scale = (1.0 / (S_H * S_W)) if fp8 else 1.0
    ones_val = (S_H * S_W) if fp8 else 1.0

    with tile.TileContext(nc) as tc, ExitStack() as ctx:
        # ---------- persistent constants ----------
        # DMA order matters: only w1/b1 (+ first x block) gate the first
        # matmul, so emit those first and defer w2/wkv into block 0's
        # compute window.
        cpool = ctx.enter_context(tc.tile_pool(name="consts", bufs=1))
        w1_sb = cpool.tile([128, 8, C], HDT, name="w1_sb")
        nc.sync.dma_start(w1_sb, w1.rearrange("(k p) m -> p k m", p=128))
        b1_sb = cpool.tile([128, 8], F32, name="b1_sb")
        nc.sync.dma_start(b1_sb, b1s)
        w2_sb = cpool.tile([128, 8, C], HDT, name="w2_sb")
        wkv_sb = cpool.tile([128, 8, 2 * C], FP16, name="wkv_sb")
        ident_sb = cpool.tile([128, 64], F32, name="ident_sb")

        def emit_deferred_consts():
            nc.sync.dma_start(w2_sb, w2.rearrange("(k p) m -> p k m", p=128))
            nc.sync.dma_start(wkv_sb, wkv.rearrange("(k p) m -> p k m", p=128))
            nc.sync.dma_start(ident_sb, ident)
        if with_bias:
            ones_sb = cpool.tile([1, 128], F32, name="ones_sb")
            nc.vector.memset(ones_sb, ones_val)
            ones_r = cpool.tile([1, 128], FP16, name="ones_r")
            nc.vector.tensor_copy(ones_r, ones_sb)
            b2_r = cpool.tile([1, C], FP16, name="b2_r")
            nc.sync.dma_start(b2_r, b2r)

        spool = ctx.enter_context(tc.tile_pool(name="spairs", bufs=1))
        spairs = [spool.tile([128, 128], FP16, name=f"spair{j}") for j in range(8)]

        # ctx accumulator in PSUM: heads 0-7 on partitions 0-63, heads
        # 8-15 on 64-127; head h at cols (h%8)*64, layout [e, d].
        ctxps_pool = ctx.enter_context(
            tc.tile_pool(name="ctxps", bufs=1, space="PSUM")
        )
        ctx_ps = ctxps_pool.tile([128, 512], F32, name="ctx_ps")

        # phase-B xq tiles live alongside phase A so DMA prefetch overlaps
        bxq_pool = ctx.enter_context(tc.tile_pool(name="bxq", bufs=6))

        def emit_bxq_dma(blk):
            bx = bxq_pool.tile([128, 8, 512], FP16, name="bx", tag="bx")
            nc.sync.dma_start(
                bx,
                xqt.rearrange("(j p) n -> p j n", p=128)[
                    :, :, blk * 512:(blk + 1) * 512
                ],
            )
            return bx

        # =========================================================
        # Phase A: gates + kv projection + ctx accumulation, fused
        # =========================================================
        with ExitStack() as pa:
            xt_pool = pa.enter_context(tc.tile_pool(name="xt", bufs=2))
            if fp8:
                xt8_pool = pa.enter_context(tc.tile_pool(name="xt8", bufs=2))
            ht_pool = pa.enter_context(tc.tile_pool(name="ht", bufs=2))
            g_pool = pa.enter_context(tc.tile_pool(name="g", bufs=3))
            kf_pool = pa.enter_context(tc.tile_pool(name="kf", bufs=3))
            vg_pool = pa.enter_context(tc.tile_pool(name="vg", bufs=3))
            gps_pool = pa.enter_context(
                tc.tile_pool(name="gps", bufs=3, space="PSUM")
            )
            kvps_pool = pa.enter_context(
                tc.tile_pool(name="kvps", bufs=2, space="PSUM")
            )

            bx_tiles = {}
            pending = []  # (kf, vg, global_chunk) awaiting ctx matmuls

            def emit_ctx(kf_t, vg_t, gc):
                # start=True marks the whole 2KB PSUM bank (per partition)
                # as pending-zero, so issue it exactly once per partition
                # half; the other heads' first writes then init via the
                # pending-zero overwrite instead of accumulating garbage.
                for h in range(H):
                    nc.tensor.matmul(
                        ctx_ps[
                            (h // 8) * 64:(h // 8) * 64 + 64,
                            (h % 8) * 64:(h % 8) * 64 + 64,
                        ],
                        vg_t[:, h * D:(h + 1) * D],
                        kf_t[:, h * D:(h + 1) * D],
                        start=(gc == 0 and h % 8 == 0),
                        stop=(gc == NCH - 1),
                        skip_group_check=True,
                    )

            for blk in range(4):
                if fp8:
                    xt8_in = xt8_pool.tile([128, 8, C], FP8, name="xt8_in", tag="xt8")
                    nc.sync.dma_start(
                        xt8_in,
                        xt8.rearrange("(k p) n -> p k n", p=128)[
                            :, :, blk * 1024:(blk + 1) * 1024
                        ],
                    )
                xt_in = xt_pool.tile([128, 8, C], FP16, name="xt_in", tag="xt")
                nc.sync.dma_start(
                    xt_in,
                    xt.rearrange("(k p) n -> p k n", p=128)[
                        :, :, blk * 1024:(blk + 1) * 1024
                    ],
                )

                # ---- gate1: hT[m-tile, n] = relu(x@w1+b1).T ----
                ht = ht_pool.tile([128, 8, C], HDT, name="ht", tag="ht")
                for m in range(8):
                    pss = [
                        gps_pool.tile([128, 512], F32, name="g1ps", tag="gps")
                        for _ in range(2)
                    ]
                    if fp8:
                        for kp in range(4):
                            lhs = w1_sb[:, 2 * kp:2 * kp + 2, m * 128:(m + 1) * 128]
                            for half in range(2):
                                nc.tensor.matmul(
                                    pss[half],
                                    lhs,
                                    xt8_in[:, 2 * kp:2 * kp + 2,
                                           half * 512:(half + 1) * 512],
                                    start=(kp == 0),
                                    stop=(kp == 3),
                                    perf_mode=DR,
                                )
                    else:
                        for k in range(8):
                            lhs = w1_sb[:, k, m * 128:(m + 1) * 128]
                            for half in range(2):
                                nc.tensor.matmul(
                                    pss[half],
                                    lhs,
                                    xt_in[:, k, half * 512:(half + 1) * 512],
                                    start=(k == 0),
                                    stop=(k == 7),
                                )
                    for half in range(2):
                        nc.scalar.activation(
                            ht[:, m, half * 512:(half + 1) * 512],
                            pss[half],
                            AF.Relu,
                            bias=b1_sb[:, m:m + 1],
                            scale=g1_scale,
                        )

                if blk == 0:
                    # w2/wkv arrive during block 0's gate1; xq prefetches after
                    emit_deferred_consts()
                # prefetch phase-B xq tiles while DMA is quiet (2 per block)
                for pf in (2 * blk, 2 * blk + 1):
                    if pf < 6:
                        bx_tiles[pf] = emit_bxq_dma(pf)

                # ---- per chunk: gate2 -> kv -> (delayed) ctx ----
                for c in range(8):
                    gc = blk * 8 + c
                    gt = g_pool.tile([128, C], FP16, name="gt", tag="gt")
                    for t in range(2):
                        ps2 = gps_pool.tile([128, 512], F32, name="g2ps", tag="gps")
                        if fp8:
                            for kp in range(4):
                                nc.tensor.matmul(
                                    ps2,
                                    ht[:, 2 * kp:2 * kp + 2, c * 128:(c + 1) * 128],
                                    w2_sb[:, 2 * kp:2 * kp + 2,
                                          t * 512:(t + 1) * 512],
                                    start=(kp == 0),
                                    stop=(kp == 3 and not with_bias),
                                    perf_mode=DR,
                                )
                        else:
                            for k in range(8):
                                nc.tensor.matmul(
                                    ps2,
                                    ht[:, k, c * 128:(c + 1) * 128],
                                    w2_sb[:, k, t * 512:(t + 1) * 512],
                                    start=(k == 0),
                                    stop=(k == 7 and not with_bias),
                                )
                        if with_bias:
                            nc.tensor.matmul(
                                ps2,
                                ones_r,
                                b2_r[:, t * 512:(t + 1) * 512],
                                start=False,
                                stop=True,
                            )
                        nc.scalar.activation(
                            gt[:, t * 512:(t + 1) * 512], ps2, AF.Sigmoid,
                            scale=g2_scale,
                        )

                    # kv projection for this chunk; k and v psum halves
                    ps_k = kvps_pool.tile([128, C], F32, name="ps_k", tag="kvps")
                    ps_v = kvps_pool.tile([128, C], F32, name="ps_v", tag="kvps")
                    for k in range(8):
                        lhs = xt_in[:, k, c * 128:(c + 1) * 128]
                        for t in range(2):
                            nc.tensor.matmul(
                                ps_k[:, t * 512:(t + 1) * 512],
                                lhs,
                                wkv_sb[:, k, t * 512:(t + 1) * 512],
                                start=(k == 0),
                                stop=(k == 7),
                            )
                        for t in range(2):
                            nc.tensor.matmul(
                                ps_v[:, t * 512:(t + 1) * 512],
                                lhs,
                                wkv_sb[:, k, C + t * 512:C + (t + 1) * 512],
                                start=(k == 0),
                                stop=(k == 7),
                            )
                    kf = kf_pool.tile([128, C], FP16, name="kf", tag="kf")
                    nc.scalar.copy(kf, ps_k)
                    vg = vg_pool.tile([128, C], FP16, name="vg", tag="vg")
                    nc.vector.tensor_mul(vg, ps_v, gt)

                    # ctx for the PREVIOUS chunk (kf/vg conversions for it
                    # ran while this chunk's kv matmuls streamed)
                    if pending:
                        emit_ctx(*pending.pop(0))
                    pending.append((kf, vg, gc))

            while pending:
                emit_ctx(*pending.pop(0))

        # =========================================================
        # Softmax over d (free dim of ctxT) + block-diag S pairs
        # =========================================================
        with ExitStack() as sm:
            smp = sm.enter_context(tc.tile_pool(name="smpool", bufs=1))
            smps = sm.enter_context(tc.tile_pool(name="smps", bufs=2, space="PSUM"))
            maxs = smp.tile([128, 8], F32, name="maxs")
            nc.vector.tensor_reduce(
                maxs,
                ctx_ps.rearrange("p (b d) -> p b d", b=8),
                axis=mybir.AxisListType.X,
                op=mybir.AluOpType.max,
            )
            cmx = smp.tile([128, 512], F32, name="cmx")
            nc.vector.tensor_sub(
                cmx.rearrange("p (h d) -> p h d", h=8),
                ctx_ps.rearrange("p (h d) -> p h d", h=8),
                maxs.unsqueeze(-1).broadcast_to([128, 8, 64]),
            )
            et = smp.tile([128, 512], F32, name="et")
            nc.scalar.activation(et, cmx, AF.Exp, scale=float(SCALE))
            sums = smp.tile([128, 8], F32, name="sums")
            nc.vector.tensor_reduce(
                sums,
                et.rearrange("p (b d) -> p b d", b=8),
                axis=mybir.AxisListType.X,
                op=mybir.AluOpType.add,
            )
            recs = smp.tile([128, 8], F32, name="recs")
            nc.vector.reciprocal(recs, sums)
            st = smp.tile([128, 512], F32, name="st")
            nc.vector.tensor_mul(
                st.rearrange("p (h d) -> p h d", h=8),
                et.rearrange("p (h d) -> p h d", h=8),
                recs.unsqueeze(-1).broadcast_to([128, 8, 64]),
            )
            # st rows e (64 per half), cols d per head.  Transposing the
            # side-by-side pair [ctxT_2j | ctxT_2j+1] ([64, 128]) gives
            # [S_2j stacked above S_2j+1] ([128, 64]); scatter block-diag.
            zero_sb = smp.tile([128, 128], FP16, name="zero_sb")
            nc.vector.memset(zero_sb, 0.0)
            for j in range(8):
                half = j // 4  # heads 0-7 in lower partitions, 8-15 upper
                base = half * 64
                colj = (2 * j) % 8
                tp = smps.tile([128, 64], F32, name="smtp", tag="smtp")
                nc.tensor.transpose(
                    tp,
                    st[base:base + 64, colj * 64:(colj + 2) * 64],
                    ident_sb[base:base + 64, :],
                )
                if j % 2 == 0:
                    nc.vector.tensor_copy(spairs[j], zero_sb)
                else:
                    nc.scalar.copy(spairs[j], zero_sb)
                if j % 2 == 0:
                    nc.vector.tensor_copy(spairs[j][0:64, 0:64], tp[0:64, :])
                    nc.vector.tensor_copy(spairs[j][64:128, 64:128], tp[64:128, :])
                else:
                    nc.scalar.copy(spairs[j][0:64, 0:64], tp[0:64, :])
                    nc.scalar.copy(spairs[j][64:128, 64:128], tp[64:128, :])

        # =========================================================
        # Phase B: o[nchunk, j*128:(j+1)*128] = q_pair @ blockdiag(S)
        # =========================================================
        with ExitStack() as pb:
            oo_pool = pb.enter_context(tc.tile_pool(name="bo", bufs=6))
            bops_pool = pb.enter_context(
                tc.tile_pool(name="bops", bufs=6, space="PSUM")
            )
            for blk in range(8):
                if blk + 6 < 8:
                    bx_tiles[blk + 6] = emit_bxq_dma(blk + 6)
                bx = bx_tiles.pop(blk)
                for c4 in range(4):
                    oo = oo_pool.tile([128, C], FP16, name="oo", tag="oo")
                    nch = blk * 4 + c4
                    for half in range(2):
                        ops = bops_pool.tile([128, 512], F32, name="ops", tag="ops")
                        for jj in range(4):
                            j = half * 4 + jj
                            nc.tensor.matmul(
                                ops[:, jj * 128:(jj + 1) * 128],
                                bx[:, j, c4 * 128:(c4 + 1) * 128],
                                spairs[j],
                                start=True,
                                stop=True,
                                skip_group_check=True,
                            )
                        if half == 0:
                            nc.vector.tensor_copy(
                                oo[:, half * 512:(half + 1) * 512], ops
                            )
                        else:
                            nc.scalar.copy(
                                oo[:, half * 512:(half + 1) * 512], ops
                            )
                        # ship each half as soon as its copy lands
                        nc.sync.dma_start(
                            o[nch * 128:(nch + 1) * 128,
                              half * 512:(half + 1) * 512],
                            oo[:, half * 512:(half + 1) * 512],
                        )

    nc.compile()
    return nc


def _get_program(gate_mode=None, with_bias=False):
    if gate_mode is None:
        gate_mode = GATE_MODE
    key = (gate_mode, bool(with_bias))
    if key not in _CACHE:
        _CACHE[key] = _build_program(gate_mode, with_bias)
    return _CACHE[key]


def make_in_maps(x1, x2, Wkv1, Wkv2, g1_w1, g1_b1, g1_w2, g1_b2,
                 g2_w1, g2_b1, g2_w2, g2_b2, gate_mode=None):
    """Core (s, b): cores 0-3 = (s=0, b), cores 4-7 = (s=1, b)."""
    import ml_dtypes
    if gate_mode is None:
        gate_mode = GATE_MODE
    fp8 = gate_mode == "fp8"
    F8 = ml_dtypes.float8_e4m3
    ident = np.vstack([np.eye(64, dtype=np.float32)] * 2)

    def prep_stream(x, wkv, w1, b1, w2, b2):
        m = {
            "xt": x.T.astype(np.float16, order="C"),
            "wkv": wkv.astype(np.float16),
            "ident": ident,
        }
        if fp8:
            m["xt8"] = (x.T * S_X).astype(F8, order="C")
            m["w1"] = (w1 * S_W).astype(F8)
            m["w2"] = (w2 * S_W).astype(F8)
            m["b1s"] = np.ascontiguousarray((S_H * b1).reshape(8, 128).T)
        else:
            m["w1"] = w1.astype(np.float16)
            m["w2"] = w2.astype(np.float16)
            m["b1s"] = np.ascontiguousarray(b1.reshape(8, 128).T)
        m["b2r"] = b2.reshape(1, C).astype(np.float16)
        return m

    in_maps = []
    for core in range(8):
        s, b = core // 4, core % 4
        if s == 0:
            m = prep_stream(x1[b], Wkv1, g1_w1, g1_b1, g1_w2, g1_b2)
            m["xqt"] = x2[b].T.astype(np.float16, order="C")
        else:
            m = prep_stream(x2[b], Wkv2, g2_w1, g2_b1, g2_w2, g2_b2)
            m["xqt"] = x1[b].T.astype(np.float16, order="C")
        in_maps.append(m)
    return in_maps


def kernel(x1, x2, Wkv1, Wkv2, g1_w1, g1_b1, g1_w2, g1_b2,
           g2_w1, g2_b1, g2_w2, g2_b2, _runner=None):
    """Full-input entry point.  Returns (o1, o2), each [4, 4096, 1024] f32."""
    from concourse.bass_utils import run_bass_kernel_spmd

    args = [np.asarray(a, dtype=np.float32) for a in
            (x1, x2, Wkv1, Wkv2, g1_w1, g1_b1, g1_w2, g1_b2,
             g2_w1, g2_b1, g2_w2, g2_b2)]
    with_bias = bool(np.any(args[7]) or np.any(args[11]))  # g1_b2, g2_b2
    nc = _get_program(GATE_MODE, with_bias)
    in_maps = make_in_maps(*args)
    if not with_bias:
        for m in in_maps:
            m.pop("b2r", None)
    if _runner is None:
        res = run_bass_kernel_spmd(nc, in_maps, core_ids=list(range(8)))
        results = res.results
    else:
        results = _runner(nc, in_maps)

    B = x1.shape[0]
    o1 = np.empty((B, N, C), dtype=np.float32)
    o2 = np.empty((B, N, C), dtype=np.float32)
    for core in range(8):
        s, b = core // 4, core % 4
        out = np.asarray(results[core]["o"], dtype=np.float32)
        if s == 0:
            o2[b] = out   # core projected x1 -> ctx1 -> o2 = q2 @ ctx1
        else:
            o1[b] = out
    return (o1, o2)


# revision 27
# speedup vs baseline: 1.6995x; 1.0104x over previous
"""Trainium2 Bass kernel for nn_CrossAttention (dense_transformer).

Reference computation (per batch b, per stream s in {1,2}):
    q_s   = heads(x_s)                      # [H, N, D] slices of x_s
    kv_s  = x_s @ Wkv_s -> k_s, v_s         # [N, C] each
    gate_s= sigmoid(relu(x_s @ w1 + b1) @ w2 + b2)
    ctx_s = softmax_d( scale * k_s^T @ (v_s * gate_s) )   # [H, D, D]
    o_1   = q_1 @ ctx_2 ; o_2 = q_2 @ ctx_1  (cross)

Sharding: 8 cores = (stream s, batch b) pairs.  Core (s, b) projects
x_s[b] (kv + gate + ctx_s[b]) and then computes the OTHER stream's
output o_{1-s}[b] = q_{1-s}[b] @ softmax(ctx_s[b]).  No cross-core
communication; host concatenates outputs.

v2: host pre-transposes/pre-casts x (fp16), so the device does no
transposes and no DRAM spills.  All GEMMs fp16 (1 cycle/row); the gate
MLP can optionally run fp8e4 DoubleRow (0.5 cycles/row).  ctx is
accumulated in PSUM across all 32 n-chunks (two 8-head groups stacked
on partition halves -> one PSUM bank).
"""

import numpy as np
from contextlib import ExitStack

N = 4096
C = 1024
H = 16
D = 64
SCALE = D ** (-0.5)
NCH = N // 128       # 32 n-chunks of 128 rows

GATE_MODE = "fp8"    # 'fp16' | 'fp8'
S_X = 16.0           # fp8 activation scale for x
S_W = 256.0          # fp8 weight scale
S_H = 32.0           # fp8 scale for hidden h

_CACHE = {}


def _build_program(gate_mode, with_bias):
    import concourse.bass as bass
    import concourse.bacc as bacc
    import concourse.tile as tile
    import concourse.mybir as mybir

    F32 = mybir.dt.float32
    FP16 = mybir.dt.float16
    FP8 = mybir.dt.float8e4
    AF = mybir.ActivationFunctionType
    DR = mybir.MatmulPerfMode.DoubleRow
    fp8 = gate_mode == "fp8"
    HDT = FP8 if fp8 else FP16

    nc = bacc.Bacc("TRN2", target_bir_lowering=False, debug=False, num_devices=8)

    # weights arrive host-rearranged to the SBUF layout [p, k, m] so the
    # DMA is a straight contiguous copy (8-16KB lines per partition)
    xt = nc.dram_tensor("xt", [C, N], FP16, kind="ExternalInput").ap()
    xqt = nc.dram_tensor("xqt", [C, N], FP16, kind="ExternalInput").ap()
    wkv = nc.dram_tensor("wkv", [128, 8 * 2 * C], FP16, kind="ExternalInput").ap()
    w1 = nc.dram_tensor("w1", [128, 8 * C], HDT, kind="ExternalInput").ap()
    w2 = nc.dram_tensor("w2", [128, 8 * C], HDT, kind="ExternalInput").ap()
    b1s = nc.dram_tensor("b1s", [128, 8], F32, kind="ExternalInput").ap()
    ident = nc.dram_tensor("ident", [128, 64], F32, kind="ExternalInput").ap()
    if fp8:
        xt8 = nc.dram_tensor("xt8", [C, N], FP8, kind="ExternalInput").ap()
    if with_bias:
        b2r = nc.dram_tensor("b2r", [1, C], FP16, kind="ExternalInput").ap()
    o = nc.dram_tensor("o", [N, C], FP16, kind="ExternalOutput").ap()

    # activation post-scales to undo the fp8 pre-scales
    g1_scale = (S_H / (S_X * S_W)) if fp8 else 1.0
    g2_scale = (1.0 / (S_H * S_W)) if fp8 else 1.0
    ones_val = (S_H * S_W) if fp8 else 1.0

    with tile.TileContext(nc) as tc, ExitStack() as ctx:
        # ---------- persistent constants ----------
        # DMA order matters: only w1/b1 (+ first x block) gate the first
        # matmul, so emit those first and defer w2/wkv into block 0's
        # compute window.
        cpool = ctx.enter_context(tc.tile_pool(name="consts", bufs=1))
        w1_sb = cpool.tile([128, 8, C], HDT, name="w1_sb")
        nc.sync.dma_start(w1_sb, w1.rearrange("p (k m) -> p k m", k=8))
        b1_sb = cpool.tile([128, 8], F32, name="b1_sb")
        nc.sync.dma_start(b1_sb, b1s)
        w2_sb = cpool.tile([128, 8, C], HDT, name="w2_sb")
        wkv_sb = cpool.tile([128, 8, 2 * C], FP16, name="wkv_sb")
        ident_sb = cpool.tile([128, 64], F32, name="ident_sb")

        def emit_deferred_consts():
            nc.sync.dma_start(w2_sb, w2.rearrange("p (k m) -> p k m", k=8))
            nc.sync.dma_start(wkv_sb, wkv.rearrange("p (k m) -> p k m", k=8))
            nc.sync.dma_start(ident_sb, ident)
        if with_bias:
            ones_sb = cpool.tile([1, 128], F32, name="ones_sb")
            nc.vector.memset(ones_sb, ones_val)
            ones_r = cpool.tile([1, 128], FP16, name="ones_r")
            nc.vector.tensor_copy(ones_r, ones_sb)
            b2_r = cpool.tile([1, C], FP16, name="b2_r")
            nc.sync.dma_start(b2_r, b2r)

        spool = ctx.enter_context(tc.tile_pool(name="spairs", bufs=1))
        spairs = [spool.tile([128, 128], FP16, name=f"spair{j}") for j in range(8)]

        # ctx accumulator in PSUM: heads 0-7 on partitions 0-63, heads
        # 8-15 on 64-127; head h at cols (h%8)*64, layout [e, d].
        ctxps_pool = ctx.enter_context(
            tc.tile_pool(name="ctxps", bufs=1, space="PSUM")
        )
        ctx_ps = ctxps_pool.tile([128, 512], F32, name="ctx_ps")

        # phase-B xq tiles live alongside phase A so DMA prefetch overlaps
        bxq_pool = ctx.enter_context(tc.tile_pool(name="bxq", bufs=2))

        def emit_bxq_dma(blk):
            bx = bxq_pool.tile([128, 8, 1024], FP16, name="bx", tag="bx")
            nc.sync.dma_start(
                bx,
                xqt.rearrange("(j p) n -> p j n", p=128)[
                    :, :, blk * 1024:(blk + 1) * 1024
                ],
            )
            return bx

        # =========================================================
        # Phase A: gates + kv projection + ctx accumulation, fused
        # =========================================================
        with ExitStack() as pa:
            xt_pool = pa.enter_context(tc.tile_pool(name="xt", bufs=2))
            if fp8:
                xt8_pool = pa.enter_context(tc.tile_pool(name="xt8", bufs=2))
            ht_pool = pa.enter_context(tc.tile_pool(name="ht", bufs=2))
            g_pool = pa.enter_context(tc.tile_pool(name="g", bufs=3))
            kf_pool = pa.enter_context(tc.tile_pool(name="kf", bufs=3))
            vg_pool = pa.enter_context(tc.tile_pool(name="vg", bufs=3))
            gps_pool = pa.enter_context(
                tc.tile_pool(name="gps", bufs=3, space="PSUM")
            )
            kvps_pool = pa.enter_context(
                tc.tile_pool(name="kvps", bufs=2, space="PSUM")
            )

            bx_tiles = {}
            pending = []  # (kf, vg, global_chunk) awaiting ctx matmuls

            def emit_ctx(kf_t, vg_t, gc):
                # start=True marks the whole 2KB PSUM bank (per partition)
                # as pending-zero, so issue it exactly once per partition
                # half; the other heads' first writes then init via the
                # pending-zero overwrite instead of accumulating garbage.
                for h in range(H):
                    nc.tensor.matmul(
                        ctx_ps[
                            (h // 8) * 64:(h // 8) * 64 + 64,
                            (h % 8) * 64:(h % 8) * 64 + 64,
                        ],
                        vg_t[:, h * D:(h + 1) * D],
                        kf_t[:, h * D:(h + 1) * D],
                        start=(gc == 0 and h % 8 == 0),
                        stop=(gc == NCH - 1),
                        skip_group_check=True,
                    )

            for blk in range(4):
                # input DMAs split by n-half so the first matmul of the
                # block is gated on half the bytes
                if fp8:
                    xt8_in = xt8_pool.tile([128, 8, C], FP8, name="xt8_in", tag="xt8")
                    for hf in range(2):
                        nc.sync.dma_start(
                            xt8_in[:, :, hf * 512:(hf + 1) * 512],
                            xt8.rearrange("(k p) n -> p k n", p=128)[
                                :, :,
                                blk * 1024 + hf * 512:blk * 1024 + (hf + 1) * 512
                            ],
                        )
                xt_in = xt_pool.tile([128, 8, C], FP16, name="xt_in", tag="xt")
                for hf in range(2):
                    nc.sync.dma_start(
                        xt_in[:, :, hf * 512:(hf + 1) * 512],
                        xt.rearrange("(k p) n -> p k n", p=128)[
                            :, :,
                            blk * 1024 + hf * 512:blk * 1024 + (hf + 1) * 512
                        ],
                    )

                # ---- gate1: hT[m-tile, n] = relu(x@w1+b1).T ----
                ht = ht_pool.tile([128, 8, C], HDT, name="ht", tag="ht")
                for m in range(8):
                    pss = [
                        gps_pool.tile([128, 512], F32, name="g1ps", tag="gps")
                        for _ in range(2)
                    ]
                    if fp8:
                        for kp in range(4):
                            lhs = w1_sb[:, 2 * kp:2 * kp + 2, m * 128:(m + 1) * 128]
                            for half in range(2):
                                nc.tensor.matmul(
                                    pss[half],
                                    lhs,
                                    xt8_in[:, 2 * kp:2 * kp + 2,
                                           half * 512:(half + 1) * 512],
                                    start=(kp == 0),
                                    stop=(kp == 3),
                                    perf_mode=DR,
                                )
                    else:
                        for k in range(8):
                            lhs = w1_sb[:, k, m * 128:(m + 1) * 128]
                            for half in range(2):
                                nc.tensor.matmul(
                                    pss[half],
                                    lhs,
                                    xt_in[:, k, half * 512:(half + 1) * 512],
                                    start=(k == 0),
                                    stop=(k == 7),
                                )
                    for half in range(2):
                        nc.scalar.activation(
                            ht[:, m, half * 512:(half + 1) * 512],
                            pss[half],
                            AF.Relu,
                            bias=b1_sb[:, m:m + 1],
                            scale=g1_scale,
                        )

                if blk == 0:
                    # w2/wkv arrive during block 0's gate1; xq prefetches after
                    emit_deferred_consts()
                # prefetch phase-B xq tiles while DMA is quiet
                if blk >= 2:
                    bx_tiles[blk - 2] = emit_bxq_dma(blk - 2)

                # ---- per chunk: gate2 -> kv -> (delayed) ctx ----
                for c in range(8):
                    gc = blk * 8 + c
                    gt = g_pool.tile([128, C], FP16, name="gt", tag="gt")
                    for t in range(2):
                        ps2 = gps_pool.tile([128, 512], F32, name="g2ps", tag="gps")
                        if fp8:
                            for kp in range(4):
                                nc.tensor.matmul(
                                    ps2,
                                    ht[:, 2 * kp:2 * kp + 2, c * 128:(c + 1) * 128],
                                    w2_sb[:, 2 * kp:2 * kp + 2,
                                          t * 512:(t + 1) * 512],
                                    start=(kp == 0),
                                    stop=(kp == 3 and not with_bias),
                                    perf_mode=DR,
                                )
                        else:
                            for k in range(8):
                                nc.tensor.matmul(
                                    ps2,
                                    ht[:, k, c * 128:(c + 1) * 128],
                                    w2_sb[:, k, t * 512:(t + 1) * 512],
                                    start=(k == 0),
                                    stop=(k == 7 and not with_bias),
                                )
                        if with_bias:
                            nc.tensor.matmul(
                                ps2,
                                ones_r,
                                b2_r[:, t * 512:(t + 1) * 512],
                                start=False,
                                stop=True,
                            )
                        nc.scalar.activation(
                            gt[:, t * 512:(t + 1) * 512], ps2, AF.Sigmoid,
                            scale=g2_scale,
                        )

                    # kv projection for this chunk; k and v psum halves
                    ps_k = kvps_pool.tile([128, C], F32, name="ps_k", tag="kvps")
                    ps_v = kvps_pool.tile([128, C], F32, name="ps_v", tag="kvps")
                    for k in range(8):
                        lhs = xt_in[:, k, c * 128:(c + 1) * 128]
                        for t in range(2):
                            nc.tensor.matmul(
                                ps_k[:, t * 512:(t + 1) * 512],
                                lhs,
                                wkv_sb[:, k, t * 512:(t + 1) * 512],
                                start=(k == 0),
                                stop=(k == 7),
                            )
                        for t in range(2):
                            nc.tensor.matmul(
                                ps_v[:, t * 512:(t + 1) * 512],
                                lhs,
                                wkv_sb[:, k, C + t * 512:C + (t + 1) * 512],
                                start=(k == 0),
                                stop=(k == 7),
                            )
                    kf = kf_pool.tile([128, C], FP16, name="kf", tag="kf")
                    nc.scalar.copy(kf, ps_k)
                    vg = vg_pool.tile([128, C], FP16, name="vg", tag="vg")
                    nc.vector.tensor_mul(vg, ps_v, gt)

                    # ctx for the PREVIOUS chunk (kf/vg conversions for it
                    # ran while this chunk's kv matmuls streamed)
                    if pending:
                        emit_ctx(*pending.pop(0))
                    pending.append((kf, vg, gc))

            while pending:
                emit_ctx(*pending.pop(0))

        # =========================================================
        # Softmax over d (free dim of ctxT) + block-diag S pairs
        # =========================================================
        with ExitStack() as sm:
            smp = sm.enter_context(tc.tile_pool(name="smpool", bufs=1))
            smps = sm.enter_context(tc.tile_pool(name="smps", bufs=2, space="PSUM"))
            maxs = smp.tile([128, 8], F32, name="maxs")
            nc.vector.tensor_reduce(
                maxs,
                ctx_ps.rearrange("p (b d) -> p b d", b=8),
                axis=mybir.AxisListType.X,
                op=mybir.AluOpType.max,
            )
            negsm = smp.tile([128, 8], F32, name="negsm")
            nc.vector.tensor_scalar_mul(negsm, maxs, -float(SCALE))
            et = smp.tile([128, 512], F32, name="et")
            for h in range(8):
                nc.scalar.activation(
                    et[:, h * 64:(h + 1) * 64],
                    ctx_ps[:, h * 64:(h + 1) * 64],
                    AF.Exp,
                    bias=negsm[:, h:h + 1],
                    scale=float(SCALE),
                )
            sums = smp.tile([128, 8], F32, name="sums")
            nc.vector.tensor_reduce(
                sums,
                et.rearrange("p (b d) -> p b d", b=8),
                axis=mybir.AxisListType.X,
                op=mybir.AluOpType.add,
            )
            recs = smp.tile([128, 8], F32, name="recs")
            nc.vector.reciprocal(recs, sums)
            st = smp.tile([128, 512], F32, name="st")
            nc.vector.tensor_mul(
                st.rearrange("p (h d) -> p h d", h=8),
                et.rearrange("p (h d) -> p h d", h=8),
                recs.unsqueeze(-1).broadcast_to([128, 8, 64]),
            )
            # st rows e (64 per half), cols d per head.  Transposing the
            # side-by-side pair [ctxT_2j | ctxT_2j+1] ([64, 128]) gives
            # [S_2j stacked above S_2j+1] ([128, 64]); scatter block-diag.
            zero_sb = smp.tile([128, 128], FP16, name="zero_sb")
            nc.vector.memset(zero_sb, 0.0)
            for j in range(8):
                half = j // 4  # heads 0-7 in lower partitions, 8-15 upper
                base = half * 64
                colj = (2 * j) % 8
                tp = smps.tile([128, 64], F32, name="smtp", tag="smtp")
                nc.tensor.transpose(
                    tp,
                    st[base:base + 64, colj * 64:(colj + 2) * 64],
                    ident_sb[base:base + 64, :],
                )
                if j % 2 == 0:
                    nc.vector.tensor_copy(spairs[j], zero_sb)
                else:
                    nc.scalar.copy(spairs[j], zero_sb)
                if j % 2 == 0:
                    nc.vector.tensor_copy(spairs[j][0:64, 0:64], tp[0:64, :])
                    nc.vector.tensor_copy(spairs[j][64:128, 64:128], tp[64:128, :])
                else:
                    nc.scalar.copy(spairs[j][0:64, 0:64], tp[0:64, :])
                    nc.scalar.copy(spairs[j][64:128, 64:128], tp[64:128, :])

        # =========================================================
        # Phase B: o[nchunk, j*128:(j+1)*128] = q_pair @ blockdiag(S)
        # =========================================================
        with ExitStack() as pb:
            oo_pool = pb.enter_context(tc.tile_pool(name="bo", bufs=6))
            bops_pool = pb.enter_context(
                tc.tile_pool(name="bops", bufs=6, space="PSUM")
            )
            for blk in range(4):
                if blk + 2 < 4:
                    bx_tiles[blk + 2] = emit_bxq_dma(blk + 2)
                bx = bx_tiles.pop(blk)
                for c4 in range(8):
                    oo = oo_pool.tile([128, C], FP16, name="oo", tag="oo")
                    nch = blk * 8 + c4
                    for half in range(2):
                        ops = bops_pool.tile([128, 512], F32, name="ops", tag="ops")
                        for jj in range(4):
                            j = half * 4 + jj
                            nc.tensor.matmul(
                                ops[:, jj * 128:(jj + 1) * 128],
                                bx[:, j, c4 * 128:(c4 + 1) * 128],
                                spairs[j],
                                start=True,
                                stop=True,
                                skip_group_check=True,
                            )
                        if half == 0:
                            nc.vector.tensor_copy(
                                oo[:, half * 512:(half + 1) * 512], ops
                            )
                        else:
                            nc.scalar.copy(
                                oo[:, half * 512:(half + 1) * 512], ops
                            )
                    nc.sync.dma_start(o[nch * 128:(nch + 1) * 128, :], oo)

    nc.compile()
    return nc


def _get_program(gate_mode=None, with_bias=False):
    if gate_mode is None:
        gate_mode = GATE_MODE
    key = (gate_mode, bool(with_bias))
    if key not in _CACHE:
        _CACHE[key] = _build_program(gate_mode, with_bias)
    return _CACHE[key]


def make_in_maps(x1, x2, Wkv1, Wkv2, g1_w1, g1_b1, g1_w2, g1_b2,
                 g2_w1, g2_b1, g2_w2, g2_b2, gate_mode=None):
    """Core (s, b): cores 0-3 = (s=0, b), cores 4-7 = (s=1, b)."""
    import ml_dtypes
    if gate_mode is None:
        gate_mode = GATE_MODE
    fp8 = gate_mode == "fp8"
    F8 = ml_dtypes.float8_e4m3
    ident = np.vstack([np.eye(64, dtype=np.float32)] * 2)

    def dev_w(w):
        # [k*128+p, m] -> [p, k*M+m] (SBUF layout, contiguous DMA lines)
        M = w.shape[1]
        return np.ascontiguousarray(
            w.reshape(8, 128, M).transpose(1, 0, 2).reshape(128, 8 * M)
        )

    def prep_stream(x, wkv, w1, b1, w2, b2):
        m = {
            "xt": x.T.astype(np.float16, order="C"),
            "wkv": dev_w(wkv.astype(np.float16)),
            "ident": ident,
        }
        if fp8:
            m["xt8"] = (x.T * S_X).astype(F8, order="C")
            m["w1"] = dev_w((w1 * S_W).astype(F8))
            m["w2"] = dev_w((w2 * S_W).astype(F8))
            m["b1s"] = np.ascontiguousarray((S_H * b1).reshape(8, 128).T)
        else:
            m["w1"] = dev_w(w1.astype(np.float16))
            m["w2"] = dev_w(w2.astype(np.float16))
            m["b1s"] = np.ascontiguousarray(b1.reshape(8, 128).T)
        m["b2r"] = b2.reshape(1, C).astype(np.float16)
        return m

    in_maps = []
    for core in range(8):
        s, b = core // 4, core % 4
        if s == 0:
            m = prep_stream(x1[b], Wkv1, g1_w1, g1_b1, g1_w2, g1_b2)
            m["xqt"] = x2[b].T.astype(np.float16, order="C")
        else:
            m = prep_stream(x2[b], Wkv2, g2_w1, g2_b1, g2_w2, g2_b2)
            m["xqt"] = x1[b].T.astype(np.float16, order="C")
        in_maps.append(m)
    return in_maps


def kernel(x1, x2, Wkv1, Wkv2, g1_w1, g1_b1, g1_w2, g1_b2,
           g2_w1, g2_b1, g2_w2, g2_b2, _runner=None):
    """Full-input entry point.  Returns (o1, o2), each [4, 4096, 1024] f32."""
    from concourse.bass_utils import run_bass_kernel_spmd

    args = [np.asarray(a, dtype=np.float32) for a in
            (x1, x2, Wkv1, Wkv2, g1_w1, g1_b1, g1_w2, g1_b2,
             g2_w1, g2_b1, g2_w2, g2_b2)]
    with_bias = bool(np.any(args[7]) or np.any(args[11]))  # g1_b2, g2_b2
    nc = _get_program(GATE_MODE, with_bias)
    in_maps = make_in_maps(*args)
    if not with_bias:
        for m in in_maps:
            m.pop("b2r", None)
    if _runner is None:
        res = run_bass_kernel_spmd(nc, in_maps, core_ids=list(range(8)))
        results = res.results
    else:
        results = _runner(nc, in_maps)

    B = x1.shape[0]
    o1 = np.empty((B, N, C), dtype=np.float32)
    o2 = np.empty((B, N, C), dtype=np.float32)
    for core in range(8):
        s, b = core // 4, core % 4
        out = np.asarray(results[core]["o"], dtype=np.float32)
        if s == 0:
            o2[b] = out   # core projected x1 -> ctx1 -> o2 = q2 @ ctx1
        else:
            o1[b] = out
    return (o1, o2)


# revision 32
# speedup vs baseline: 1.7379x; 1.0226x over previous
"""Trainium2 Bass kernel for nn_CrossAttention (dense_transformer).

Reference computation (per batch b, per stream s in {1,2}):
    q_s   = heads(x_s)                      # [H, N, D] slices of x_s
    kv_s  = x_s @ Wkv_s -> k_s, v_s         # [N, C] each
    gate_s= sigmoid(relu(x_s @ w1 + b1) @ w2 + b2)
    ctx_s = softmax_d( scale * k_s^T @ (v_s * gate_s) )   # [H, D, D]
    o_1   = q_1 @ ctx_2 ; o_2 = q_2 @ ctx_1  (cross)

Sharding: 8 cores = (stream s, batch b) pairs.  Core (s, b) projects
x_s[b] (kv + gate + ctx_s[b]) and then computes the OTHER stream's
output o_{1-s}[b] = q_{1-s}[b] @ softmax(ctx_s[b]).  No cross-core
communication; host concatenates outputs.

v2: host pre-transposes/pre-casts x (fp16), so the device does no
transposes and no DRAM spills.  All GEMMs fp16 (1 cycle/row); the gate
MLP can optionally run fp8e4 DoubleRow (0.5 cycles/row).  ctx is
accumulated in PSUM across all 32 n-chunks (two 8-head groups stacked
on partition halves -> one PSUM bank).
"""

import numpy as np
from contextlib import ExitStack

N = 4096
C = 1024
H = 16
D = 64
SCALE = D ** (-0.5)
NCH = N // 128       # 32 n-chunks of 128 rows

GATE_MODE = "fp8"    # 'fp16' | 'fp8'
S_X = 16.0           # fp8 activation scale for x
S_W = 256.0          # fp8 weight scale
S_H = 32.0           # fp8 scale for hidden h

_CACHE = {}


def _build_program(gate_mode, with_bias):
    import concourse.bass as bass
    import concourse.bacc as bacc
    import concourse.tile as tile
    import concourse.mybir as mybir

    F32 = mybir.dt.float32
    FP16 = mybir.dt.float16
    FP8 = mybir.dt.float8e4
    AF = mybir.ActivationFunctionType
    DR = mybir.MatmulPerfMode.DoubleRow
    fp8 = gate_mode == "fp8"
    HDT = FP8 if fp8 else FP16

    nc = bacc.Bacc("TRN2", target_bir_lowering=False, debug=False, num_devices=8)

    # weights arrive host-rearranged to the SBUF layout [p, k, m] so the
    # DMA is a straight contiguous copy (8-16KB lines per partition)
    xt = nc.dram_tensor("xt", [C, N], FP16, kind="ExternalInput").ap()
    xqt = nc.dram_tensor("xqt", [C, N], FP16, kind="ExternalInput").ap()
    wkv = nc.dram_tensor("wkv", [128, 8 * 2 * C], FP16, kind="ExternalInput").ap()
    w1 = nc.dram_tensor("w1", [128, 8 * C], HDT, kind="ExternalInput").ap()
    w2 = nc.dram_tensor("w2", [128, 8 * C], HDT, kind="ExternalInput").ap()
    b1s = nc.dram_tensor("b1s", [128, 8], F32, kind="ExternalInput").ap()
    ident = nc.dram_tensor("ident", [128, 64], F32, kind="ExternalInput").ap()
    if fp8:
        xt8 = nc.dram_tensor("xt8", [C, N], FP8, kind="ExternalInput").ap()
    if with_bias:
        b2r = nc.dram_tensor("b2r", [1, C], FP16, kind="ExternalInput").ap()
    o = nc.dram_tensor("o", [N, C], FP16, kind="ExternalOutput").ap()

    # activation post-scales to undo the fp8 pre-scales
    g1_scale = (S_H / (S_X * S_W)) if fp8 else 1.0
    g2_scale = (1.0 / (S_H * S_W)) if fp8 else 1.0
    ones_val = (S_H * S_W) if fp8 else 1.0

    with tile.TileContext(nc) as tc, ExitStack() as ctx:
        # ---------- persistent constants ----------
        # DMA order matters: only w1/b1 (+ first x block) gate the first
        # matmul, so emit those first and defer w2/wkv into block 0's
        # compute window.
        # big DMAs are split into k-range parts so they spread across DMA
        # queues (a single dma_start lands on one queue)
        def dma_split(dst, src, parts):
            kk = 8 // parts
            for i in range(parts):
                nc.sync.dma_start(
                    dst[:, i * kk:(i + 1) * kk, :], src[:, i * kk:(i + 1) * kk, :]
                )

        cpool = ctx.enter_context(tc.tile_pool(name="consts", bufs=1))
        w1_sb = cpool.tile([128, 8, C], HDT, name="w1_sb")
        dma_split(w1_sb, w1.rearrange("p (k m) -> p k m", k=8), 4)
        b1_sb = cpool.tile([128, 8], F32, name="b1_sb")
        nc.sync.dma_start(b1_sb, b1s)
        w2_sb = cpool.tile([128, 8, C], HDT, name="w2_sb")
        wkv_sb = cpool.tile([128, 8, 2 * C], FP16, name="wkv_sb")
        ident_sb = cpool.tile([128, 64], F32, name="ident_sb")

        def emit_deferred_consts():
            dma_split(w2_sb, w2.rearrange("p (k m) -> p k m", k=8), 4)
            dma_split(wkv_sb, wkv.rearrange("p (k m) -> p k m", k=8), 8)
            nc.sync.dma_start(ident_sb, ident)
        if with_bias:
            ones_sb = cpool.tile([1, 128], F32, name="ones_sb")
            nc.vector.memset(ones_sb, ones_val)
            ones_r = cpool.tile([1, 128], FP16, name="ones_r")
            nc.vector.tensor_copy(ones_r, ones_sb)
            b2_r = cpool.tile([1, C], FP16, name="b2_r")
            nc.sync.dma_start(b2_r, b2r)

        spool = ctx.enter_context(tc.tile_pool(name="spairs", bufs=1))
        spairs = [spool.tile([128, 128], FP16, name=f"spair{j}") for j in range(8)]

        # ctx accumulator in PSUM: heads 0-7 on partitions 0-63, heads
        # 8-15 on 64-127; head h at cols (h%8)*64, layout [e, d].
        ctxps_pool = ctx.enter_context(
            tc.tile_pool(name="ctxps", bufs=1, space="PSUM")
        )
        ctx_ps = ctxps_pool.tile([128, 512], F32, name="ctx_ps")

        # phase-B xq tiles live alongside phase A so DMA prefetch overlaps
        bxq_pool = ctx.enter_context(tc.tile_pool(name="bxq", bufs=3))

        def emit_bxq_dma(blk):
            bx = bxq_pool.tile([128, 8, 1024], FP16, name="bx", tag="bx")
            src = xqt.rearrange("(j p) n -> p j n", p=128)[
                :, :, blk * 1024:(blk + 1) * 1024
            ]
            for i in range(4):
                nc.sync.dma_start(bx[:, 2 * i:2 * i + 2, :], src[:, 2 * i:2 * i + 2, :])
            return bx

        # =========================================================
        # Phase A: gates + kv projection + ctx accumulation, fused
        # =========================================================
        with ExitStack() as pa:
            xt_pool = pa.enter_context(tc.tile_pool(name="xt", bufs=2))
            if fp8:
                xt8_pool = pa.enter_context(tc.tile_pool(name="xt8", bufs=2))
            ht_pool = pa.enter_context(tc.tile_pool(name="ht", bufs=2))
            g_pool = pa.enter_context(tc.tile_pool(name="g", bufs=3))
            kf_pool = pa.enter_context(tc.tile_pool(name="kf", bufs=3))
            vg_pool = pa.enter_context(tc.tile_pool(name="vg", bufs=3))
            gps_pool = pa.enter_context(
                tc.tile_pool(name="gps", bufs=3, space="PSUM")
            )
            kvps_pool = pa.enter_context(
                tc.tile_pool(name="kvps", bufs=2, space="PSUM")
            )

            bx_tiles = {}
            pending = []  # (kf, vg, global_chunk) awaiting ctx matmuls

            def emit_ctx(kf_t, vg_t, gc):
                # start=True marks the whole 2KB PSUM bank (per partition)
                # as pending-zero, so issue it exactly once per partition
                # half; the other heads' first writes then init via the
                # pending-zero overwrite instead of accumulating garbage.
                for h in range(H):
                    nc.tensor.matmul(
                        ctx_ps[
                            (h // 8) * 64:(h // 8) * 64 + 64,
                            (h % 8) * 64:(h % 8) * 64 + 64,
                        ],
                        vg_t[:, h * D:(h + 1) * D],
                        kf_t[:, h * D:(h + 1) * D],
                        start=(gc == 0 and h % 8 == 0),
                        stop=(gc == NCH - 1),
                        skip_group_check=True,
                    )

            for blk in range(4):
                # input DMAs split by n-half so the first matmul of the
                # block is gated on half the bytes
                if fp8:
                    xt8_in = xt8_pool.tile([128, 8, C], FP8, name="xt8_in", tag="xt8")
                    src8 = xt8.rearrange("(k p) n -> p k n", p=128)
                    for hf in range(2):
                        for kq in range(2):
                            nc.sync.dma_start(
                                xt8_in[:, kq * 4:(kq + 1) * 4,
                                       hf * 512:(hf + 1) * 512],
                                src8[:, kq * 4:(kq + 1) * 4,
                                     blk * 1024 + hf * 512:
                                     blk * 1024 + (hf + 1) * 512],
                            )
                xt_in = xt_pool.tile([128, 8, C], FP16, name="xt_in", tag="xt")
                srcx = xt.rearrange("(k p) n -> p k n", p=128)
                for hf in range(2):
                    for kq in range(2):
                        nc.sync.dma_start(
                            xt_in[:, kq * 4:(kq + 1) * 4,
                                  hf * 512:(hf + 1) * 512],
                            srcx[:, kq * 4:(kq + 1) * 4,
                                 blk * 1024 + hf * 512:
                                 blk * 1024 + (hf + 1) * 512],
                        )

                # ---- gate1: hT[m-tile, n] = relu(x@w1+b1).T ----
                ht = ht_pool.tile([128, 8, C], HDT, name="ht", tag="ht")
                for m in range(8):
                    pss = [
                        gps_pool.tile([128, 512], F32, name="g1ps", tag="gps")
                        for _ in range(2)
                    ]
                    if fp8:
                        for kp in range(4):
                            lhs = w1_sb[:, 2 * kp:2 * kp + 2, m * 128:(m + 1) * 128]
                            for half in range(2):
                                nc.tensor.matmul(
                                    pss[half],
                                    lhs,
                                    xt8_in[:, 2 * kp:2 * kp + 2,
                                           half * 512:(half + 1) * 512],
                                    start=(kp == 0),
                                    stop=(kp == 3),
                                    perf_mode=DR,
                                )
                    else:
                        for k in range(8):
                            lhs = w1_sb[:, k, m * 128:(m + 1) * 128]
                            for half in range(2):
                                nc.tensor.matmul(
                                    pss[half],
                                    lhs,
                                    xt_in[:, k, half * 512:(half + 1) * 512],
                                    start=(k == 0),
                                    stop=(k == 7),
                                )
                    for half in range(2):
                        nc.scalar.activation(
                            ht[:, m, half * 512:(half + 1) * 512],
                            pss[half],
                            AF.Relu,
                            bias=b1_sb[:, m:m + 1],
                            scale=g1_scale,
                        )

                if blk == 0:
                    # w2/wkv arrive during block 0's gate1; xq prefetches after
                    emit_deferred_consts()
                # prefetch phase-B xq tiles while DMA is quiet
                if blk >= 1:
                    bx_tiles[blk - 1] = emit_bxq_dma(blk - 1)

                # ---- per chunk: gate2 -> kv -> (delayed) ctx ----
                for c in range(8):
                    gc = blk * 8 + c
                    gt = g_pool.tile([128, C], FP16, name="gt", tag="gt")
                    for t in range(2):
                        ps2 = gps_pool.tile([128, 512], F32, name="g2ps", tag="gps")
                        if fp8:
                            for kp in range(4):
                                nc.tensor.matmul(
                                    ps2,
                                    ht[:, 2 * kp:2 * kp + 2, c * 128:(c + 1) * 128],
                                    w2_sb[:, 2 * kp:2 * kp + 2,
                                          t * 512:(t + 1) * 512],
                                    start=(kp == 0),
                                    stop=(kp == 3 and not with_bias),
                                    perf_mode=DR,
                                )
                        else:
                            for k in range(8):
                                nc.tensor.matmul(
                                    ps2,
                                    ht[:, k, c * 128:(c + 1) * 128],
                                    w2_sb[:, k, t * 512:(t + 1) * 512],
                                    start=(k == 0),
                                    stop=(k == 7 and not with_bias),
                                )
                        if with_bias:
                            nc.tensor.matmul(
                                ps2,
                                ones_r,
                                b2_r[:, t * 512:(t + 1) * 512],
                                start=False,
                                stop=True,
                            )
                        nc.scalar.activation(
                            gt[:, t * 512:(t + 1) * 512], ps2, AF.Sigmoid,
                            scale=g2_scale,
                        )

                    # kv projection for this chunk; k and v psum halves
                    ps_k = kvps_pool.tile([128, C], F32, name="ps_k", tag="kvps")
                    ps_v = kvps_pool.tile([128, C], F32, name="ps_v", tag="kvps")
                    for k in range(8):
                        lhs = xt_in[:, k, c * 128:(c + 1) * 128]
                        for t in range(2):
                            nc.tensor.matmul(
                                ps_k[:, t * 512:(t + 1) * 512],
                                lhs,
                                wkv_sb[:, k, t * 512:(t + 1) * 512],
                                start=(k == 0),
                                stop=(k == 7),
                            )
                        for t in range(2):
                            nc.tensor.matmul(
                                ps_v[:, t * 512:(t + 1) * 512],
                                lhs,
                                wkv_sb[:, k, C + t * 512:C + (t + 1) * 512],
                                start=(k == 0),
                                stop=(k == 7),
                            )
                    kf = kf_pool.tile([128, C], FP16, name="kf", tag="kf")
                    nc.scalar.copy(kf, ps_k)
                    vg = vg_pool.tile([128, C], FP16, name="vg", tag="vg")
                    nc.vector.tensor_mul(vg, ps_v, gt)

                    # ctx for the PREVIOUS chunk (kf/vg conversions for it
                    # ran while this chunk's kv matmuls streamed)
                    if pending:
                        emit_ctx(*pending.pop(0))
                    pending.append((kf, vg, gc))

            while pending:
                emit_ctx(*pending.pop(0))

        # =========================================================
        # Softmax over d (free dim of ctxT) + block-diag S pairs
        # =========================================================
        with ExitStack() as sm:
            smp = sm.enter_context(tc.tile_pool(name="smpool", bufs=1))
            smps = sm.enter_context(tc.tile_pool(name="smps", bufs=2, space="PSUM"))
            maxs = smp.tile([128, 8], F32, name="maxs")
            nc.vector.tensor_reduce(
                maxs,
                ctx_ps.rearrange("p (b d) -> p b d", b=8),
                axis=mybir.AxisListType.X,
                op=mybir.AluOpType.max,
            )
            negsm = smp.tile([128, 8], F32, name="negsm")
            nc.vector.tensor_scalar_mul(negsm, maxs, -float(SCALE))
            et = smp.tile([128, 512], F32, name="et")
            for h in range(8):
                nc.scalar.activation(
                    et[:, h * 64:(h + 1) * 64],
                    ctx_ps[:, h * 64:(h + 1) * 64],
                    AF.Exp,
                    bias=negsm[:, h:h + 1],
                    scale=float(SCALE),
                )
            sums = smp.tile([128, 8], F32, name="sums")
            nc.vector.tensor_reduce(
                sums,
                et.rearrange("p (b d) -> p b d", b=8),
                axis=mybir.AxisListType.X,
                op=mybir.AluOpType.add,
            )
            recs = smp.tile([128, 8], F32, name="recs")
            nc.vector.reciprocal(recs, sums)
            st = smp.tile([128, 512], F32, name="st")
            nc.vector.tensor_mul(
                st.rearrange("p (h d) -> p h d", h=8),
                et.rearrange("p (h d) -> p h d", h=8),
                recs.unsqueeze(-1).broadcast_to([128, 8, 64]),
            )
            # st rows e (64 per half), cols d per head.  Transposing the
            # side-by-side pair [ctxT_2j | ctxT_2j+1] ([64, 128]) gives
            # [S_2j stacked above S_2j+1] ([128, 64]); scatter block-diag.
            zero_sb = smp.tile([128, 128], FP16, name="zero_sb")
            nc.vector.memset(zero_sb, 0.0)
            for j in range(8):
                half = j // 4  # heads 0-7 in lower partitions, 8-15 upper
                base = half * 64
                colj = (2 * j) % 8
                tp = smps.tile([128, 64], F32, name="smtp", tag="smtp")
                nc.tensor.transpose(
                    tp,
                    st[base:base + 64, colj * 64:(colj + 2) * 64],
                    ident_sb[base:base + 64, :],
                )
                if j % 2 == 0:
                    nc.vector.tensor_copy(spairs[j], zero_sb)
                else:
                    nc.scalar.copy(spairs[j], zero_sb)
                if j % 2 == 0:
                    nc.vector.tensor_copy(spairs[j][0:64, 0:64], tp[0:64, :])
                    nc.vector.tensor_copy(spairs[j][64:128, 64:128], tp[64:128, :])
                else:
                    nc.scalar.copy(spairs[j][0:64, 0:64], tp[0:64, :])
                    nc.scalar.copy(spairs[j][64:128, 64:128], tp[64:128, :])

        # =========================================================
        # Phase B: o[nchunk, j*128:(j+1)*128] = q_pair @ blockdiag(S)
        # =========================================================
        with ExitStack() as pb:
            oo_pool = pb.enter_context(tc.tile_pool(name="bo", bufs=6))
            bops_pool = pb.enter_context(
                tc.tile_pool(name="bops", bufs=6, space="PSUM")
            )
            for blk in range(4):
                if blk + 3 < 4:
                    bx_tiles[blk + 3] = emit_bxq_dma(blk + 3)
                bx = bx_tiles.pop(blk)
                for c4 in range(8):
                    oo = oo_pool.tile([128, C], FP16, name="oo", tag="oo")
                    nch = blk * 8 + c4
                    for half in range(2):
                        ops = bops_pool.tile([128, 512], F32, name="ops", tag="ops")
                        for jj in range(4):
                            j = half * 4 + jj
                            nc.tensor.matmul(
                                ops[:, jj * 128:(jj + 1) * 128],
                                bx[:, j, c4 * 128:(c4 + 1) * 128],
                                spairs[j],
                                start=True,
                                stop=True,
                                skip_group_check=True,
                            )
                        if half == 0:
                            nc.vector.tensor_copy(
                                oo[:, half * 512:(half + 1) * 512], ops
                            )
                        else:
                            nc.scalar.copy(
                                oo[:, half * 512:(half + 1) * 512], ops
                            )
                    nc.sync.dma_start(o[nch * 128:(nch + 1) * 128, :], oo)

    nc.compile()
    return nc


def _get_program(gate_mode=None, with_bias=False):
    if gate_mode is None:
        gate_mode = GATE_MODE
    key = (gate_mode, bool(with_bias))
    if key not in _CACHE:
        _CACHE[key] = _build_program(gate_mode, with_bias)
    return _CACHE[key]


def make_in_maps(x1, x2, Wkv1, Wkv2, g1_w1, g1_b1, g1_w2, g1_b2,
                 g2_w1, g2_b1, g2_w2, g2_b2, gate_mode=None):
    """Core (s, b): cores 0-3 = (s=0, b), cores 4-7 = (s=1, b)."""
    import ml_dtypes
    if gate_mode is None:
        gate_mode = GATE_MODE
    fp8 = gate_mode == "fp8"
    F8 = ml_dtypes.float8_e4m3
    ident = np.vstack([np.eye(64, dtype=np.float32)] * 2)

    def dev_w(w):
        # [k*128+p, m] -> [p, k*M+m] (SBUF layout, contiguous DMA lines)
        M = w.shape[1]
        return np.ascontiguousarray(
            w.reshape(8, 128, M).transpose(1, 0, 2).reshape(128, 8 * M)
        )

    def prep_stream(x, wkv, w1, b1, w2, b2):
        m = {
            "xt": x.T.astype(np.float16, order="C"),
            "wkv": dev_w(wkv.astype(np.float16)),
            "ident": ident,
        }
        if fp8:
            m["xt8"] = (x.T * S_X).astype(F8, order="C")
            m["w1"] = dev_w((w1 * S_W).astype(F8))
            m["w2"] = dev_w((w2 * S_W).astype(F8))
            m["b1s"] = np.ascontiguousarray((S_H * b1).reshape(8, 128).T)
        else:
            m["w1"] = dev_w(w1.astype(np.float16))
            m["w2"] = dev_w(w2.astype(np.float16))
            m["b1s"] = np.ascontiguousarray(b1.reshape(8, 128).T)
        m["b2r"] = b2.reshape(1, C).astype(np.float16)
        return m

    in_maps = []
    for core in range(8):
        s, b = core // 4, core % 4
        if s == 0:
            m = prep_stream(x1[b], Wkv1, g1_w1, g1_b1, g1_w2, g1_b2)
            m["xqt"] = x2[b].T.astype(np.float16, order="C")
        else:
            m = prep_stream(x2[b], Wkv2, g2_w1, g2_b1, g2_w2, g2_b2)
            m["xqt"] = x1[b].T.astype(np.float16, order="C")
        in_maps.append(m)
    return in_maps


def kernel(x1, x2, Wkv1, Wkv2, g1_w1, g1_b1, g1_w2, g1_b2,
           g2_w1, g2_b1, g2_w2, g2_b2, _runner=None):
    """Full-input entry point.  Returns (o1, o2), each [4, 4096, 1024] f32."""
    from concourse.bass_utils import run_bass_kernel_spmd

    args = [np.asarray(a, dtype=np.float32) for a in
            (x1, x2, Wkv1, Wkv2, g1_w1, g1_b1, g1_w2, g1_b2,
             g2_w1, g2_b1, g2_w2, g2_b2)]
    with_bias = bool(np.any(args[7]) or np.any(args[11]))  # g1_b2, g2_b2
    nc = _get_program(GATE_MODE, with_bias)
    in_maps = make_in_maps(*args)
    if not with_bias:
        for m in in_maps:
            m.pop("b2r", None)
    if _runner is None:
        res = run_bass_kernel_spmd(nc, in_maps, core_ids=list(range(8)))
        results = res.results
    else:
        results = _runner(nc, in_maps)

    B = x1.shape[0]
    o1 = np.empty((B, N, C), dtype=np.float32)
    o2 = np.empty((B, N, C), dtype=np.float32)
    for core in range(8):
        s, b = core // 4, core % 4
        out = np.asarray(results[core]["o"], dtype=np.float32)
        if s == 0:
            o2[b] = out   # core projected x1 -> ctx1 -> o2 = q2 @ ctx1
        else:
            o1[b] = out
    return (o1, o2)


# revision 38
# speedup vs baseline: 1.7748x; 1.0212x over previous
"""Trainium2 Bass kernel for nn_CrossAttention (dense_transformer).

Reference computation (per batch b, per stream s in {1,2}):
    q_s   = heads(x_s)                      # [H, N, D] slices of x_s
    kv_s  = x_s @ Wkv_s -> k_s, v_s         # [N, C] each
    gate_s= sigmoid(relu(x_s @ w1 + b1) @ w2 + b2)
    ctx_s = softmax_d( scale * k_s^T @ (v_s * gate_s) )   # [H, D, D]
    o_1   = q_1 @ ctx_2 ; o_2 = q_2 @ ctx_1  (cross)

Sharding: 8 cores = (stream s, batch b) pairs.  Core (s, b) projects
x_s[b] (kv + gate + ctx_s[b]) and then computes the OTHER stream's
output o_{1-s}[b] = q_{1-s}[b] @ softmax(ctx_s[b]).  No cross-core
communication; host concatenates outputs.

v2: host pre-transposes/pre-casts x (fp16), so the device does no
transposes and no DRAM spills.  All GEMMs fp16 (1 cycle/row); the gate
MLP can optionally run fp8e4 DoubleRow (0.5 cycles/row).  ctx is
accumulated in PSUM across all 32 n-chunks (two 8-head groups stacked
on partition halves -> one PSUM bank).
"""

import numpy as np
from contextlib import ExitStack

N = 4096
C = 1024
H = 16
D = 64
SCALE = D ** (-0.5)
NCH = N // 128       # 32 n-chunks of 128 rows

GATE_MODE = "fp8"    # 'fp16' | 'fp8'
S_X = 16.0           # fp8 activation scale for x
S_W = 256.0          # fp8 weight scale
S_H = 32.0           # fp8 scale for hidden h

_CACHE = {}


def _build_program(gate_mode, with_bias):
    import concourse.bass as bass
    import concourse.bacc as bacc
    import concourse.tile as tile
    import concourse.mybir as mybir

    F32 = mybir.dt.float32
    FP16 = mybir.dt.float16
    FP8 = mybir.dt.float8e4
    AF = mybir.ActivationFunctionType
    DR = mybir.MatmulPerfMode.DoubleRow
    fp8 = gate_mode == "fp8"
    HDT = FP8 if fp8 else FP16

    nc = bacc.Bacc("TRN2", target_bir_lowering=False, debug=False, num_devices=8)

    # weights arrive host-rearranged to the SBUF layout [p, k, m] so the
    # DMA is a straight contiguous copy (8-16KB lines per partition)
    xt = nc.dram_tensor("xt", [C, N], FP16, kind="ExternalInput").ap()
    xqt = nc.dram_tensor("xqt", [C, N], FP16, kind="ExternalInput").ap()
    wkv = nc.dram_tensor("wkv", [128, 8 * 2 * C], FP16, kind="ExternalInput").ap()
    w1 = nc.dram_tensor("w1", [128, 8 * C], HDT, kind="ExternalInput").ap()
    w2 = nc.dram_tensor("w2", [128, 8 * C], HDT, kind="ExternalInput").ap()
    b1s = nc.dram_tensor("b1s", [128, 8], F32, kind="ExternalInput").ap()
    ident = nc.dram_tensor("ident", [128, 64], F32, kind="ExternalInput").ap()
    if fp8:
        xt8 = nc.dram_tensor("xt8", [C, N], FP8, kind="ExternalInput").ap()
    if with_bias:
        b2r = nc.dram_tensor("b2r", [1, C], FP16, kind="ExternalInput").ap()
    o = nc.dram_tensor("o", [N, C], FP16, kind="ExternalOutput").ap()

    # activation post-scales to undo the fp8 pre-scales
    g1_scale = (S_H / (S_X * S_W)) if fp8 else 1.0
    g2_scale = (1.0 / (S_H * S_W)) if fp8 else 1.0
    ones_val = (S_H * S_W) if fp8 else 1.0

    with tile.TileContext(nc) as tc, ExitStack() as ctx:
        # ---------- persistent constants ----------
        # DMA order matters: only w1/b1 (+ first x block) gate the first
        # matmul, so emit those first and defer w2/wkv into block 0's
        # compute window.
        # big DMAs are split into k-range parts so they spread across DMA
        # queues (a single dma_start lands on one queue)
        def dma_split(dst, src, parts):
            kk = 8 // parts
            for i in range(parts):
                nc.sync.dma_start(
                    dst[:, i * kk:(i + 1) * kk, :], src[:, i * kk:(i + 1) * kk, :]
                )

        cpool = ctx.enter_context(tc.tile_pool(name="consts", bufs=1))
        w1_sb = cpool.tile([128, 8, C], HDT, name="w1_sb")
        dma_split(w1_sb, w1.rearrange("p (k m) -> p k m", k=8), 8)
        b1_sb = cpool.tile([128, 8], F32, name="b1_sb")
        nc.sync.dma_start(b1_sb, b1s)
        w2_sb = cpool.tile([128, 8, C], HDT, name="w2_sb")
        wkv_sb = cpool.tile([128, 8, 2 * C], FP16, name="wkv_sb")
        ident_sb = cpool.tile([128, 64], F32, name="ident_sb")

        def emit_deferred_consts():
            dma_split(w2_sb, w2.rearrange("p (k m) -> p k m", k=8), 4)
            dma_split(wkv_sb, wkv.rearrange("p (k m) -> p k m", k=8), 8)
            nc.sync.dma_start(ident_sb, ident)
        if with_bias:
            ones_sb = cpool.tile([1, 128], F32, name="ones_sb")
            nc.vector.memset(ones_sb, ones_val)
            ones_r = cpool.tile([1, 128], FP16, name="ones_r")
            nc.vector.tensor_copy(ones_r, ones_sb)
            b2_r = cpool.tile([1, C], FP16, name="b2_r")
            nc.sync.dma_start(b2_r, b2r)

        spool = ctx.enter_context(tc.tile_pool(name="spairs", bufs=1))
        spairs = [spool.tile([128, 128], FP16, name=f"spair{j}") for j in range(8)]

        # ctx accumulator in PSUM: heads 0-7 on partitions 0-63, heads
        # 8-15 on 64-127; head h at cols (h%8)*64, layout [e, d].
        ctxps_pool = ctx.enter_context(
            tc.tile_pool(name="ctxps", bufs=1, space="PSUM")
        )
        ctx_ps = ctxps_pool.tile([128, 512], F32, name="ctx_ps")

        # phase-B xq tiles live alongside phase A so DMA prefetch overlaps
        bxq_pool = ctx.enter_context(tc.tile_pool(name="bxq", bufs=4))

        def emit_bxq_dma(blk):
            bx = bxq_pool.tile([128, 8, 1024], FP16, name="bx", tag="bx")
            src = xqt.rearrange("(j p) n -> p j n", p=128)[
                :, :, blk * 1024:(blk + 1) * 1024
            ]
            for i in range(4):
                nc.sync.dma_start(bx[:, 2 * i:2 * i + 2, :], src[:, 2 * i:2 * i + 2, :])
            return bx

        # =========================================================
        # Phase A: gates + kv projection + ctx accumulation, fused
        # =========================================================
        with ExitStack() as pa:
            xt_pool = pa.enter_context(tc.tile_pool(name="xt", bufs=2))
            if fp8:
                xt8_pool = pa.enter_context(tc.tile_pool(name="xt8", bufs=2))
            ht_pool = pa.enter_context(tc.tile_pool(name="ht", bufs=2))
            g_pool = pa.enter_context(tc.tile_pool(name="g", bufs=3))
            kf_pool = pa.enter_context(tc.tile_pool(name="kf", bufs=3))
            vg_pool = pa.enter_context(tc.tile_pool(name="vg", bufs=3))
            gps_pool = pa.enter_context(
                tc.tile_pool(name="gps", bufs=3, space="PSUM")
            )
            kvps_pool = pa.enter_context(
                tc.tile_pool(name="kvps", bufs=2, space="PSUM")
            )

            bx_tiles = {}
            pending = []  # (kf, vg, global_chunk) awaiting ctx matmuls

            def emit_ctx(kf_t, vg_t, gc):
                # start=True marks the whole 2KB PSUM bank (per partition)
                # as pending-zero, so issue it exactly once per partition
                # half; the other heads' first writes then init via the
                # pending-zero overwrite instead of accumulating garbage.
                for h in range(H):
                    nc.tensor.matmul(
                        ctx_ps[
                            (h // 8) * 64:(h // 8) * 64 + 64,
                            (h % 8) * 64:(h % 8) * 64 + 64,
                        ],
                        vg_t[:, h * D:(h + 1) * D],
                        kf_t[:, h * D:(h + 1) * D],
                        start=(gc == 0 and h % 8 == 0),
                        stop=(gc == NCH - 1),
                        skip_group_check=True,
                    )

            for blk in range(4):
                # input DMAs split by n-half so the first matmul of the
                # block is gated on half the bytes
                if fp8:
                    xt8_in = xt8_pool.tile([128, 8, C], FP8, name="xt8_in", tag="xt8")
                    src8 = xt8.rearrange("(k p) n -> p k n", p=128)
                    for hf in range(2):
                        for kq in range(4):
                            nc.sync.dma_start(
                                xt8_in[:, kq * 2:(kq + 1) * 2,
                                       hf * 512:(hf + 1) * 512],
                                src8[:, kq * 2:(kq + 1) * 2,
                                     blk * 1024 + hf * 512:
                                     blk * 1024 + (hf + 1) * 512],
                            )
                xt_in = xt_pool.tile([128, 8, C], FP16, name="xt_in", tag="xt")
                srcx = xt.rearrange("(k p) n -> p k n", p=128)
                for hf in range(2):
                    for kq in range(2):
                        nc.sync.dma_start(
                            xt_in[:, kq * 4:(kq + 1) * 4,
                                  hf * 512:(hf + 1) * 512],
                            srcx[:, kq * 4:(kq + 1) * 4,
                                 blk * 1024 + hf * 512:
                                 blk * 1024 + (hf + 1) * 512],
                        )

                # ---- gate1: hT[m-tile, n] = relu(x@w1+b1).T ----
                ht = ht_pool.tile([128, 8, C], HDT, name="ht", tag="ht")
                for m in range(8):
                    pss = [
                        gps_pool.tile([128, 512], F32, name="g1ps", tag="gps")
                        for _ in range(2)
                    ]
                    if fp8:
                        for kp in range(4):
                            lhs = w1_sb[:, 2 * kp:2 * kp + 2, m * 128:(m + 1) * 128]
                            for half in range(2):
                                nc.tensor.matmul(
                                    pss[half],
                                    lhs,
                                    xt8_in[:, 2 * kp:2 * kp + 2,
                                           half * 512:(half + 1) * 512],
                                    start=(kp == 0),
                                    stop=(kp == 3),
                                    perf_mode=DR,
                                )
                    else:
                        for k in range(8):
                            lhs = w1_sb[:, k, m * 128:(m + 1) * 128]
                            for half in range(2):
                                nc.tensor.matmul(
                                    pss[half],
                                    lhs,
                                    xt_in[:, k, half * 512:(half + 1) * 512],
                                    start=(k == 0),
                                    stop=(k == 7),
                                )
                    for half in range(2):
                        nc.scalar.activation(
                            ht[:, m, half * 512:(half + 1) * 512],
                            pss[half],
                            AF.Relu,
                            bias=b1_sb[:, m:m + 1],
                            scale=g1_scale,
                        )

                if blk == 0:
                    # w2/wkv arrive during block 0's gate1; xq prefetches after
                    emit_deferred_consts()
                # prefetch phase-B xq tiles while DMA is quiet
                if blk >= 1:
                    bx_tiles[blk - 1] = emit_bxq_dma(blk - 1)
                    if blk == 3:
                        bx_tiles[3] = emit_bxq_dma(3)

                # ---- per chunk: gate2 -> kv -> (delayed) ctx ----
                for c in range(8):
                    gc = blk * 8 + c
                    gt = g_pool.tile([128, C], FP16, name="gt", tag="gt")
                    for t in range(2):
                        ps2 = gps_pool.tile([128, 512], F32, name="g2ps", tag="gps")
                        if fp8:
                            for kp in range(4):
                                nc.tensor.matmul(
                                    ps2,
                                    ht[:, 2 * kp:2 * kp + 2, c * 128:(c + 1) * 128],
                                    w2_sb[:, 2 * kp:2 * kp + 2,
                                          t * 512:(t + 1) * 512],
                                    start=(kp == 0),
                                    stop=(kp == 3 and not with_bias),
                                    perf_mode=DR,
                                )
                        else:
                            for k in range(8):
                                nc.tensor.matmul(
                                    ps2,
                                    ht[:, k, c * 128:(c + 1) * 128],
                                    w2_sb[:, k, t * 512:(t + 1) * 512],
                                    start=(k == 0),
                                    stop=(k == 7 and not with_bias),
                                )
                        if with_bias:
                            nc.tensor.matmul(
                                ps2,
                                ones_r,
                                b2_r[:, t * 512:(t + 1) * 512],
                                start=False,
                                stop=True,
                            )
                        nc.scalar.activation(
                            gt[:, t * 512:(t + 1) * 512], ps2, AF.Sigmoid,
                            scale=g2_scale,
                        )

                    # kv projection for this chunk; k and v psum halves
                    ps_k = kvps_pool.tile([128, C], F32, name="ps_k", tag="kvps")
                    ps_v = kvps_pool.tile([128, C], F32, name="ps_v", tag="kvps")
                    for k in range(8):
                        lhs = xt_in[:, k, c * 128:(c + 1) * 128]
                        for t in range(2):
                            nc.tensor.matmul(
                                ps_k[:, t * 512:(t + 1) * 512],
                                lhs,
                                wkv_sb[:, k, t * 512:(t + 1) * 512],
                                start=(k == 0),
                                stop=(k == 7),
                            )
                        for t in range(2):
                            nc.tensor.matmul(
                                ps_v[:, t * 512:(t + 1) * 512],
                                lhs,
                                wkv_sb[:, k, C + t * 512:C + (t + 1) * 512],
                                start=(k == 0),
                                stop=(k == 7),
                            )
                    kf = kf_pool.tile([128, C], FP16, name="kf", tag="kf")
                    nc.scalar.copy(kf, ps_k)
                    vg = vg_pool.tile([128, C], FP16, name="vg", tag="vg")
                    nc.vector.tensor_mul(vg, ps_v, gt)

                    # ctx for the PREVIOUS chunk (kf/vg conversions for it
                    # ran while this chunk's kv matmuls streamed)
                    if pending:
                        emit_ctx(*pending.pop(0))
                    pending.append((kf, vg, gc))

            while pending:
                emit_ctx(*pending.pop(0))

        # =========================================================
        # Softmax over d (free dim of ctxT) + block-diag S pairs
        # =========================================================
        with ExitStack() as sm:
            smp = sm.enter_context(tc.tile_pool(name="smpool", bufs=1))
            smps = sm.enter_context(tc.tile_pool(name="smps", bufs=2, space="PSUM"))
            maxs = smp.tile([128, 8], F32, name="maxs")
            nc.vector.tensor_reduce(
                maxs,
                ctx_ps.rearrange("p (b d) -> p b d", b=8),
                axis=mybir.AxisListType.X,
                op=mybir.AluOpType.max,
            )
            negsm = smp.tile([128, 8], F32, name="negsm")
            nc.vector.tensor_scalar_mul(negsm, maxs, -float(SCALE))
            et = smp.tile([128, 512], F32, name="et")
            for h in range(8):
                nc.scalar.activation(
                    et[:, h * 64:(h + 1) * 64],
                    ctx_ps[:, h * 64:(h + 1) * 64],
                    AF.Exp,
                    bias=negsm[:, h:h + 1],
                    scale=float(SCALE),
                )
            sums = smp.tile([128, 8], F32, name="sums")
            nc.vector.tensor_reduce(
                sums,
                et.rearrange("p (b d) -> p b d", b=8),
                axis=mybir.AxisListType.X,
                op=mybir.AluOpType.add,
            )
            recs = smp.tile([128, 8], F32, name="recs")
            nc.vector.reciprocal(recs, sums)
            st = smp.tile([128, 512], F32, name="st")
            nc.vector.tensor_mul(
                st.rearrange("p (h d) -> p h d", h=8),
                et.rearrange("p (h d) -> p h d", h=8),
                recs.unsqueeze(-1).broadcast_to([128, 8, 64]),
            )
            # st rows e (64 per half), cols d per head.  Transposing the
            # side-by-side pair [ctxT_2j | ctxT_2j+1] ([64, 128]) gives
            # [S_2j stacked above S_2j+1] ([128, 64]); scatter block-diag.
            zero_sb = smp.tile([128, 128], FP16, name="zero_sb")
            nc.vector.memset(zero_sb, 0.0)
            for j in range(8):
                half = j // 4  # heads 0-7 in lower partitions, 8-15 upper
                base = half * 64
                colj = (2 * j) % 8
                tp = smps.tile([128, 64], F32, name="smtp", tag="smtp")
                nc.tensor.transpose(
                    tp,
                    st[base:base + 64, colj * 64:(colj + 2) * 64],
                    ident_sb[base:base + 64, :],
                )
                if j % 2 == 0:
                    nc.vector.tensor_copy(spairs[j], zero_sb)
                else:
                    nc.scalar.copy(spairs[j], zero_sb)
                if j % 2 == 0:
                    nc.vector.tensor_copy(spairs[j][0:64, 0:64], tp[0:64, :])
                    nc.vector.tensor_copy(spairs[j][64:128, 64:128], tp[64:128, :])
                else:
                    nc.scalar.copy(spairs[j][0:64, 0:64], tp[0:64, :])
                    nc.scalar.copy(spairs[j][64:128, 64:128], tp[64:128, :])

        # =========================================================
        # Phase B: o[nchunk, j*128:(j+1)*128] = q_pair @ blockdiag(S)
        # =========================================================
        with ExitStack() as pb:
            oo_pool = pb.enter_context(tc.tile_pool(name="bo", bufs=6))
            bops_pool = pb.enter_context(
                tc.tile_pool(name="bops", bufs=6, space="PSUM")
            )
            for blk in range(4):
                if blk + 3 < 4:
                    bx_tiles[blk + 3] = emit_bxq_dma(blk + 3)
                bx = bx_tiles.pop(blk)
                for c4 in range(8):
                    oo = oo_pool.tile([128, C], FP16, name="oo", tag="oo")
                    nch = blk * 8 + c4
                    for half in range(2):
                        ops = bops_pool.tile([128, 512], F32, name="ops", tag="ops")
                        for jj in range(4):
                            j = half * 4 + jj
                            nc.tensor.matmul(
                                ops[:, jj * 128:(jj + 1) * 128],
                                bx[:, j, c4 * 128:(c4 + 1) * 128],
                                spairs[j],
                                start=True,
                                stop=True,
                                skip_group_check=True,
                            )
                        if half == 0:
                            nc.vector.tensor_copy(
                                oo[:, half * 512:(half + 1) * 512], ops
                            )
                        else:
                            nc.scalar.copy(
                                oo[:, half * 512:(half + 1) * 512], ops
                            )
                    nc.sync.dma_start(o[nch * 128:(nch + 1) * 128, :], oo)

    nc.compile()
    return nc


def _get_program(gate_mode=None, with_bias=False):
    if gate_mode is None:
        gate_mode = GATE_MODE
    key = (gate_mode, bool(with_bias))
    if key not in _CACHE:
        _CACHE[key] = _build_program(gate_mode, with_bias)
    return _CACHE[key]


def make_in_maps(x1, x2, Wkv1, Wkv2, g1_w1, g1_b1, g1_w2, g1_b2,
                 g2_w1, g2_b1, g2_w2, g2_b2, gate_mode=None):
    """Core (s, b): cores 0-3 = (s=0, b), cores 4-7 = (s=1, b)."""
    import ml_dtypes
    if gate_mode is None:
        gate_mode = GATE_MODE
    fp8 = gate_mode == "fp8"
    F8 = ml_dtypes.float8_e4m3
    ident = np.vstack([np.eye(64, dtype=np.float32)] * 2)

    def dev_w(w):
        # [k*128+p, m] -> [p, k*M+m] (SBUF layout, contiguous DMA lines)
        M = w.shape[1]
        return np.ascontiguousarray(
            w.reshape(8, 128, M).transpose(1, 0, 2).reshape(128, 8 * M)
        )

    def prep_stream(x, wkv, w1, b1, w2, b2):
        m = {
            "xt": x.T.astype(np.float16, order="C"),
            "wkv": dev_w(wkv.astype(np.float16)),
            "ident": ident,
        }
        if fp8:
            m["xt8"] = (x.T * S_X).astype(F8, order="C")
            m["w1"] = dev_w((w1 * S_W).astype(F8))
            m["w2"] = dev_w((w2 * S_W).astype(F8))
            m["b1s"] = np.ascontiguousarray((S_H * b1).reshape(8, 128).T)
        else:
            m["w1"] = dev_w(w1.astype(np.float16))
            m["w2"] = dev_w(w2.astype(np.float16))
            m["b1s"] = np.ascontiguousarray(b1.reshape(8, 128).T)
        m["b2r"] = b2.reshape(1, C).astype(np.float16)
        return m

    in_maps = []
    for core in range(8):
        s, b = core // 4, core % 4
        if s == 0:
            m = prep_stream(x1[b], Wkv1, g1_w1, g1_b1, g1_w2, g1_b2)
            m["xqt"] = x2[b].T.astype(np.float16, order="C")
        else:
            m = prep_stream(x2[b], Wkv2, g2_w1, g2_b1, g2_w2, g2_b2)
            m["xqt"] = x1[b].T.astype(np.float16, order="C")
        in_maps.append(m)
    return in_maps


def kernel(x1, x2, Wkv1, Wkv2, g1_w1, g1_b1, g1_w2, g1_b2,
           g2_w1, g2_b1, g2_w2, g2_b2, _runner=None):
    """Full-input entry point.  Returns (o1, o2), each [4, 4096, 1024] f32."""
    from concourse.bass_utils import run_bass_kernel_spmd

    args = [np.asarray(a, dtype=np.float32) for a in
            (x1, x2, Wkv1, Wkv2, g1_w1, g1_b1, g1_w2, g1_b2,
             g2_w1, g2_b1, g2_w2, g2_b2)]
    with_bias = bool(np.any(args[7]) or np.any(args[11]))  # g1_b2, g2_b2
    nc = _get_program(GATE_MODE, with_bias)
    in_maps = make_in_maps(*args)
    if not with_bias:
        for m in in_maps:
            m.pop("b2r", None)
    if _runner is None:
        res = run_bass_kernel_spmd(nc, in_maps, core_ids=list(range(8)))
        results = res.results
    else:
        results = _runner(nc, in_maps)

    B = x1.shape[0]
    o1 = np.empty((B, N, C), dtype=np.float32)
    o2 = np.empty((B, N, C), dtype=np.float32)
    for core in range(8):
        s, b = core // 4, core % 4
        out = np.asarray(results[core]["o"], dtype=np.float32)
        if s == 0:
            o2[b] = out   # core projected x1 -> ctx1 -> o2 = q2 @ ctx1
        else:
            o1[b] = out
    return (o1, o2)
